# revision 14
# baseline (speedup 1.0000x reference)
"""BiGCN (nn_BiGCN_52716428591487) Trainium2 kernel, v3.

Math: the model's output is log_softmax(cat(l2_bu[root], l2_td[root]) @ W_lin + b).
Only the layer-2 GCN values AT THE ROOT NODES matter, and GCNConv is linear in
its input features, so the whole network collapses to:

  agg1_d[v]  = sum_{e -> v} coef_d(e) * x[nbr(e)]            (v in S; self-loops
               folded into the edge list with coef dinv_d[v]^2)
  l1_d[v]    = agg1_d[v] @ W_d1 + b_d1
  out2_l1[g] = sum_{s in S_g} Pr[s, g] * relu(l1_d[s])       (layer-2 agg)
  out2_R[g]  = c_g * relu(x[root_g])                          (root-feature block
               collapses: Pr is block-diagonal by graph, c_g = sum_s Pr[s, g])
  pb/pt[g]   = relu(W2_d^T [out2_R; out2_l1_d] + b_2)
  out[g]     = log_softmax([pb, pt][g] @ W_lin + b_lin)

where S = {sources of root-incident edges} + {roots} (~1.7k of 50k nodes).

Host does index-only preprocessing (degrees, edge selection, dedup/gather
tables, the one-hot scatter matrices, Pr) plus dtype casts/layout; the device
does every arithmetic op that touches x or the weights.

Device shape: per (chunk, dir), stage-1 computes aggT = xt^T @ onehot
directly in PSUM (lhsT = gathered x rows, rhs = one-hot with coefs), so no
transposes are needed; gathered rows are deduplicated across BOTH directions
per chunk ([td-only | shared | bu-only] tile layout, one fp8 copy of x rows
serving both one-hots); l1 is a single fp8 DoubleRow matmul; the head is a
short chain of tiny matmuls + activations on [G_cap]-wide data.
W1/W2 ship as fp8 (x16 host cast-scale, descaled inside the relu
activations); one-hot coefs are x8.  5 input DMAs per core.

Sharding: graph-data parallel over 8 cores; the host concatenates the
per-core [G_cap, C] outputs.
"""

import numpy as np

P = 128
NCORES = 8
NS = 2           # slot chunks of 128 target slots per core
SC_OH = 8.0      # host scale on one-hot coefs
SC_W = 16.0      # host scale on fp8 W1/W2


def _roundup(a, m):
    return -(-int(a) // m) * m


# ----------------------------------------------------------------------------
# Host preprocessing: index-only work + gather/pack tables
# ----------------------------------------------------------------------------

def _preprocess(x, edge_index, batch, num_graphs):
    import concourse.mybir as mybir

    f8 = mybir.dt.np(mybir.dt.float8e4)

    x = np.ascontiguousarray(np.asarray(x), dtype=np.float32)
    ei = np.asarray(edge_index)
    batch = np.asarray(batch).astype(np.int64)
    G = int(np.asarray(num_graphs))
    N, F = x.shape
    nF = F // P
    src = ei[0].astype(np.int64)
    dst = ei[1].astype(np.int64)

    assert np.all(np.diff(batch) >= 0), "batch must be sorted (contiguous graphs)"
    roots = np.searchsorted(batch, np.arange(G, dtype=np.int64))  # segment_min

    deg_td = 1.0 + np.bincount(dst, minlength=N).astype(np.float64)
    deg_bu = 1.0 + np.bincount(src, minlength=N).astype(np.float64)
    dinv_td = (1.0 / np.sqrt(deg_td)).astype(np.float32)
    dinv_bu = (1.0 / np.sqrt(deg_bu)).astype(np.float32)

    G_cap = max(-(-G // NCORES), 1)

    # S: sources of root-incident edges + roots
    is_root = np.zeros(N, bool)
    is_root[roots] = True
    rmask = is_root[dst]
    r_src, r_dst = src[rmask], dst[rmask]
    r_coef = dinv_td[r_src] * dinv_td[r_dst]

    s_nodes = np.unique(np.concatenate([r_src, roots]))  # sorted
    s_graph = batch[s_nodes]
    s_count_g = np.bincount(s_graph, minlength=G)
    assert s_count_g.max() <= P, "graph S-set exceeds one chunk"

    in_s = np.zeros(N, bool)
    in_s[s_nodes] = True

    def _dir_edges(tgt_nodes, row_nodes, dinv):
        m = in_s[tgt_nodes]
        tg, rw = tgt_nodes[m], row_nodes[m]
        cf = (dinv[rw] * dinv[tg]).astype(np.float32)
        tg = np.concatenate([tg, s_nodes])          # self loops
        rw = np.concatenate([rw, s_nodes])
        cf = np.concatenate([cf, (dinv[s_nodes] ** 2).astype(np.float32)])
        return tg, rw, cf, batch[tg]

    E_td = _dir_edges(dst, src, dinv_td)
    E_bu = _dir_edges(src, dst, dinv_bu)

    # per-graph unique-row category counts (td-only / shared / bu-only)
    cat_g = np.zeros((G, 3), np.int64)
    uniq_rows_td = [None] * G
    uniq_rows_bu = [None] * G

    def _per_graph_rows(E):
        tg, rw, cf, eg = E
        order = np.argsort(eg, kind="stable")
        egs, rws = eg[order], rw[order]
        bnd = np.searchsorted(egs, np.arange(G + 1))
        return [np.unique(rws[bnd[g]:bnd[g + 1]]) for g in range(G)]

    uniq_rows_td = _per_graph_rows(E_td)
    uniq_rows_bu = _per_graph_rows(E_bu)
    for g in range(G):
        sh = np.intersect1d(uniq_rows_td[g], uniq_rows_bu[g],
                            assume_unique=True)
        cat_g[g, 1] = len(sh)
        cat_g[g, 0] = len(uniq_rows_td[g]) - len(sh)
        cat_g[g, 2] = len(uniq_rows_bu[g]) - len(sh)

    gw = cat_g.sum(axis=1)

    # graph -> (core, chunk) bin: direct greedy over NCORES*NS bins on
    # per-category loads, then local-search refinement minimizing the global
    # per-category maxima (which set the padded tile counts U/V/W).
    NB = NCORES * NS
    bin_of_graph = np.empty(G, np.int64)
    bcount = np.zeros(NB, np.int64)       # graphs per bin (core cap G_cap)
    bfill = np.zeros(NB, np.int64)        # slots per bin (cap P)
    bld = np.zeros((NB, 3))
    ccount = np.zeros(NCORES, np.int64)

    def _core_ok(b, extra=1):
        return ccount[b // NS] + extra <= G_cap * 1  # per-core graph cap

    for g in np.argsort(-gw, kind="stable"):
        cands = [b for b in range(NB)
                 if bfill[b] + s_count_g[g] <= P and ccount[b // NS] < G_cap]
        b = min(cands, key=lambda bb: ((bld[bb] + cat_g[g]).max(),
                                       bld[bb].sum()))
        bin_of_graph[g] = b
        bcount[b] += 1
        bfill[b] += s_count_g[g]
        bld[b] += cat_g[g]
        ccount[b // NS] += 1

    def _cost(loads):
        mx = loads.max(axis=0)
        tiles = np.ceil(mx / P).sum()
        return (tiles, mx.sum())

    # local search: single moves + pairwise swaps
    for _sweep in range(6):
        improved = False
        order = np.argsort(-(bld.max(axis=1)))
        for g in range(G):
            b0 = bin_of_graph[g]
            for b1 in range(NB):
                if b1 == b0:
                    continue
                if bfill[b1] + s_count_g[g] <= P and \
                        (b1 // NS == b0 // NS or ccount[b1 // NS] < G_cap):
                    new = bld.copy()
                    new[b0] -= cat_g[g]
                    new[b1] += cat_g[g]
                    if _cost(new) < _cost(bld):
                        bld = new
                        bfill[b0] -= s_count_g[g]
                        bfill[b1] += s_count_g[g]
                        ccount[b0 // NS] -= 1
                        ccount[b1 // NS] += 1
                        bcount[b0] -= 1
                        bcount[b1] += 1
                        bin_of_graph[g] = b1
                        b0 = b1
                        improved = True
        # pairwise swaps
        for g1 in range(G):
            for g2 in range(g1 + 1, G):
                b1, b2 = bin_of_graph[g1], bin_of_graph[g2]
                if b1 == b2:
                    continue
                if bfill[b1] - s_count_g[g1] + s_count_g[g2] > P or \
                        bfill[b2] - s_count_g[g2] + s_count_g[g1] > P:
                    continue
                new = bld.copy()
                new[b1] += cat_g[g2] - cat_g[g1]
                new[b2] += cat_g[g1] - cat_g[g2]
                if _cost(new) < _cost(bld):
                    bld = new
                    bfill[b1] += s_count_g[g2] - s_count_g[g1]
                    bfill[b2] += s_count_g[g1] - s_count_g[g2]
                    bin_of_graph[g1], bin_of_graph[g2] = b2, b1
                    improved = True
        if not improved:
            break

    core_of_graph = bin_of_graph // NS
    chunk_of_graph = bin_of_graph % NS
    glocal = np.empty(G, np.int64)
    counts = np.zeros(NCORES, np.int64)
    for g in range(G):
        glocal[g] = counts[core_of_graph[g]]
        counts[core_of_graph[g]] += 1

    # slot layout per bin
    s_lookup = np.full(N, -1, np.int64)   # node -> core slot (chunk*P + i)
    binfill = np.zeros(NB, np.int64)
    for g in range(G):
        b = bin_of_graph[g]
        s = chunk_of_graph[g]
        idx = s_nodes[s_graph == g]
        s_lookup[idx] = s * P + binfill[b] + np.arange(len(idx))
        binfill[b] += s_count_g[g]
    assert (binfill <= P).all()

    # ---- per (core, chunk): union rows in [td_only | shared | bu_only]
    # category layout; per dir entry lists reference row positions
    rows_cat = {}   # (c, s) -> (rows_to, rows_sh, rows_bo)
    for c in range(NCORES):
        for s in range(NS):
            gs = np.flatnonzero((core_of_graph == c) & (chunk_of_graph == s))
            rt = (np.concatenate([uniq_rows_td[g] for g in gs])
                  if len(gs) else np.empty(0, np.int64))
            rb = (np.concatenate([uniq_rows_bu[g] for g in gs])
                  if len(gs) else np.empty(0, np.int64))
            sh = np.intersect1d(rt, rb, assume_unique=True)
            to = np.setdiff1d(rt, sh, assume_unique=True)
            bo = np.setdiff1d(rb, sh, assume_unique=True)
            rows_cat[(c, s)] = (to, sh, bo)

    # uniform category tile counts; U even, V+W even (DR pair alignment)
    U = _roundup(max(len(rows_cat[k][0]) for k in rows_cat), P) // P
    V = _roundup(max(len(rows_cat[k][1]) for k in rows_cat), P) // P
    W = _roundup(max(len(rows_cat[k][2]) for k in rows_cat), P) // P
    U += U % 2
    W += (V + W) % 2
    T = U + V + W                                  # xt tiles per chunk (even)
    A_td = U + V                                   # td tile range [0, A_td)
    PA = -(-A_td // 2)                             # xt pairs in piece A

    # entry maps per (c, s, d): (row_pos_in_chunk_layout, tgt_local, coef)
    ents = {}
    for d, E in (("td", E_td), ("bu", E_bu)):
        tg, rw, cf, eg = E
        slot = s_lookup[tg]
        assert (slot >= 0).all()
        core = core_of_graph[eg]
        chunk = slot // P
        for c in range(NCORES):
            for s in range(NS):
                m = (core == c) & (chunk == s)
                rw_m, cf_m = rw[m], cf[m]
                tloc = (slot[m] - s * P).astype(np.int64)
                to, sh, bo = rows_cat[(c, s)]
                # map node id -> row position in the chunk layout
                pos = np.full(len(rw_m), -1, np.int64)
                for base, cat in ((0, to), (U * P, sh), ((U + V) * P, bo)):
                    idx = np.searchsorted(cat, rw_m)
                    idx_c = np.clip(idx, 0, max(len(cat) - 1, 0))
                    hit = (len(cat) > 0) & (cat[idx_c] == rw_m) if len(cat) \
                        else np.zeros(len(rw_m), bool)
                    pos = np.where(hit, base + idx_c, pos)
                assert (pos >= 0).all()
                ents[(c, s, d)] = (pos, tloc, cf_m)

    # layer-2 aggregation matrix Pr[core, slot, glocal] and root tables
    r_graph = batch[r_dst]
    S_cap = NS * P
    Pr = np.zeros((NCORES, S_cap, G_cap), np.float32)
    np.add.at(Pr, (core_of_graph[r_graph], s_lookup[r_src], glocal[r_graph]),
              r_coef)
    np.add.at(Pr, (core_of_graph[np.arange(G)], s_lookup[roots], glocal),
              dinv_td[roots] ** 2)

    # ---- pack per-core pieces
    # piece A (per chunk): oh_td blocks + xt pairs [0, PA)
    # piece B: oh_bu blocks + xt pairs [PA, T/2)
    def _pack(c, s):
        to, sh, bo = rows_cat[(c, s)]
        # x rows in chunk layout [T*P, F] fp8
        xg = np.zeros((T * P, F), f8)
        xg[:len(to)] = x[to].astype(f8)
        xg[U * P:U * P + len(sh)] = x[sh].astype(f8)
        xg[(U + V) * P:(U + V) * P + len(bo)] = x[bo].astype(f8)
        xg = xg.reshape(T, P, F)
        # xt pair blocks [P, nF, 2, P] -> [P, nF*2*P] per pair
        xp = xg.reshape(T // 2, 2, P, nF, P).transpose(2, 0, 3, 1, 4) \
            .reshape(P, T // 2, nF * 2 * P)

        def _oh(d, tile_lo, n_tiles):
            posm, tloc, cf = ents[(c, s, d)]
            oh = np.zeros((n_tiles, P, P), np.float32)
            t_idx = posm // P - tile_lo
            assert (t_idx >= 0).all() and (t_idx < n_tiles).all()
            np.add.at(oh, (t_idx, posm % P, tloc), cf * SC_OH)
            return oh.astype(f8)

        # td: tiles [0, A_td): pairs + possible odd single
        oh_td = _oh("td", 0, A_td)
        td_blocks = []
        for j in range(A_td // 2):
            td_blocks.append(np.stack([oh_td[2 * j], oh_td[2 * j + 1]],
                                      axis=1).reshape(P, 2 * P))
        if A_td % 2:
            td_blocks.append(oh_td[A_td - 1])
        # bu: tiles [U, T): (V+W) even -> all pairs
        oh_bu = _oh("bu", U, V + W)
        bu_blocks = []
        for j in range((V + W) // 2):
            bu_blocks.append(np.stack([oh_bu[2 * j], oh_bu[2 * j + 1]],
                                      axis=1).reshape(P, 2 * P))
        pieceA = np.concatenate(td_blocks + [xp[:, :PA].reshape(P, -1)], axis=1)
        pieceB = np.concatenate(bu_blocks + [xp[:, PA:].reshape(P, -1)], axis=1)
        return (np.ascontiguousarray(pieceA), np.ascontiguousarray(pieceB))

    in_maps = []
    for c in range(NCORES):
        m = {}
        for s in range(NS):
            a, b = _pack(c, s)
            m[f"pa{s}"] = a
            m[f"pb{s}"] = b
        m["pr"] = Pr[c]
        gs = np.flatnonzero(core_of_graph == c)
        xrootT = np.zeros((F, G_cap), np.float32)
        xrootT[:, glocal[gs]] = x[roots[gs]].T
        m["xrootT"] = xrootT
        m["croot"] = np.tile(Pr[c].sum(axis=0, dtype=np.float64)
                             .astype(np.float32), (P, 1))
        in_maps.append(m)

    meta = dict(F=F, U=U, V=V, W=W, G_cap=G_cap, counts=counts, G=G,
                core_of_graph=core_of_graph, glocal=glocal)
    return in_maps, meta


def _c16_layout(F, H, C, G_cap, bz):
    """Column layout of the bf16 constant matrix [P, W16]."""
    off = 0
    L = {}

    def add(name, w):
        nonlocal off
        L[name] = (off, w)
        off += w

    for f in range(2 * H // P):
        add(f"wl{f}", C)
    if not bz:
        add("b1td", H)
        add("b1bu", H)
        add("bl", C)
        add("ones", P)
        add("b2bu", 1)
        add("b2td", 1)
    for s in range(NS):
        add(f"pr{s}", G_cap)
    for f in range(F // P):
        add(f"xrootT{f}", G_cap)
    add("croot", G_cap)
    return L, off


def _w8_layout(F, H):
    """Column layout of the fp8 scaled-weight block (rides in piece pa0)."""
    off = 0
    L = {}

    def add(name, w):
        nonlocal off
        L[name] = (off, w)
        off += w

    add("w1td", 2 * H)            # [P, 2, H] f-interleaved pair for DR
    add("w1bu", 2 * H)
    for d in ("bu", "td"):
        for j in range((F + H) // P):
            add(f"w2{d}{j}", H)   # [P, H] chunks as lhsT
    return L, off


def _pack_consts(in_maps, inputs, meta):
    import concourse.mybir as mybir
    import ml_dtypes

    f8 = mybir.dt.np(mybir.dt.float8e4)
    bf16 = ml_dtypes.bfloat16
    H = int(np.asarray(inputs["W_td1"]).shape[1])
    C = int(np.asarray(inputs["W_lin"]).shape[1])
    F, G_cap = meta["F"], meta["G_cap"]
    bz = all(not np.any(np.asarray(inputs[k]))
             for k in ("b_td1", "b_bu1", "b_bu2", "b_td2", "b_lin"))
    L16, W16 = _c16_layout(F, H, C, G_cap, bz)
    L8, W8 = _w8_layout(F, H)
    g = lambda k: np.asarray(inputs[k], dtype=np.float32)

    base16 = np.zeros((P, W16), bf16)

    def put(name, block):
        o, w = L16[name]
        base16[:, o:o + w][tuple(slice(s) for s in block.shape)] = \
            block.astype(bf16)

    for f in range(2 * H // P):
        put(f"wl{f}", g("W_lin")[f * P:(f + 1) * P, :])
    if not bz:
        put("b1td", g("b_td1").reshape(1, H))
        put("b1bu", g("b_bu1").reshape(1, H))
        put("bl", g("b_lin").reshape(1, C))
        put("ones", np.ones((1, P), np.float32))
        put("b2bu", g("b_bu2")[:P, None])
        put("b2td", g("b_td2")[:P, None])

    w8 = np.zeros((P, W8), f8)

    def put8(name, block):
        o, w = L8[name]
        w8[:, o:o + w] = block.astype(f8)

    for d, wn in (("td", "W_td1"), ("bu", "W_bu1")):
        w1 = g(wn) * SC_W                       # [F, H] = [2P, H]
        put8(f"w1{d}", w1.reshape(2, P, H).transpose(1, 0, 2).reshape(P, 2 * H))
    for d, wn in (("bu", "W_bu2"), ("td", "W_td2")):
        w2 = g(wn) * SC_W                       # [F+H, H]
        for j in range((F + H) // P):
            put8(f"w2{d}{j}", w2[j * P:(j + 1) * P, :])

    for m in in_maps:
        c16 = base16.copy()
        pr = m.pop("pr")
        for s in range(NS):
            o, w = L16[f"pr{s}"]
            c16[:, o:o + w] = pr[s * P:(s + 1) * P].astype(bf16)
        xrootT = m.pop("xrootT")
        for f in range(F // P):
            o, w = L16[f"xrootT{f}"]
            c16[:, o:o + w] = xrootT[f * P:(f + 1) * P].astype(bf16)
        o, w = L16["croot"]
        c16[:, o:o + w] = m.pop("croot").astype(bf16)
        m["c16"] = np.ascontiguousarray(c16)
        m["pa0"] = np.ascontiguousarray(np.concatenate([w8, m["pa0"]], axis=1))

    meta["H"], meta["C"] = H, C
    meta["bz"] = bz
    return H


# ----------------------------------------------------------------------------
# Device program
# ----------------------------------------------------------------------------

def _build_program(F, H, C, U, V, W, G_cap, bz=False, repeat=1):
    from contextlib import ExitStack

    import concourse.bacc as bacc
    import concourse.bass as bass  # noqa: F401
    import concourse.mybir as mybir
    import concourse.tile as tile

    dt = mybir.dt
    f32, bf, f8 = dt.float32, dt.bfloat16, dt.float8e4
    nF = F // P
    nW2 = (F + H) // P
    assert F % P == 0 and H == P and nF == 2
    L16, W16 = _c16_layout(F, H, C, G_cap, bz)
    L8, W8 = _w8_layout(F, H)

    T = U + V + W
    A_td = U + V
    PA = -(-A_td // 2)
    XPW = nF * 2 * P                    # xt pair block cols
    # piece A: oh_td ((A_td//2) pair blocks + odd single) + xt pairs [0, PA)
    A_OH = (A_td // 2) * 2 * P + (A_td % 2) * P
    ACOLS = A_OH + PA * XPW
    # piece B: oh_bu ((V+W)//2 pair blocks) + xt pairs [PA, T/2)
    B_OH = ((V + W) // 2) * 2 * P
    BCOLS = B_OH + (T // 2 - PA) * XPW

    mul, sub, addop, maxop = (
        mybir.AluOpType.mult, mybir.AluOpType.subtract,
        mybir.AluOpType.add, mybir.AluOpType.max)
    Relu, Exp, Ln, Copy = (mybir.ActivationFunctionType.Relu,
                           mybir.ActivationFunctionType.Exp,
                           mybir.ActivationFunctionType.Ln,
                           mybir.ActivationFunctionType.Copy)
    DR = mybir.MatmulPerfMode.DoubleRow

    nc = bacc.Bacc("TRN2", target_bir_lowering=False, debug=False,
                   num_devices=NCORES)

    piece_d = {}
    for s in range(NS):
        piece_d[("a", s)] = nc.dram_tensor(
            f"pa{s}", [P, ACOLS + (W8 if s == 0 else 0)], f8,
            kind="ExternalInput").ap()
        piece_d[("b", s)] = nc.dram_tensor(
            f"pb{s}", [P, BCOLS], f8, kind="ExternalInput").ap()
    c16_d = nc.dram_tensor("c16", [P, W16], bf, kind="ExternalInput").ap()
    out_d = nc.dram_tensor("out", [G_cap, C], f32, kind="ExternalOutput").ap()

    with ExitStack() as ctx:
        tc = ctx.enter_context(tile.TileContext(nc))
        const = ctx.enter_context(tc.tile_pool(
            name="cst", bufs=(1 if repeat == 1 else 2)))
        ppool = ctx.enter_context(tc.tile_pool(name="pp", bufs=2))
        spool = ctx.enter_context(tc.tile_pool(name="sp", bufs=2))
        psA = ctx.enter_context(tc.tile_pool(name="psA", bufs=2, space="PSUM"))
        psB = ctx.enter_context(tc.tile_pool(name="psB", bufs=2, space="PSUM"))
        psO = ctx.enter_context(tc.tile_pool(name="psO", bufs=1, space="PSUM"))

        # load the one act table containing Exp/Ln/Relu/Copy up-front
        from concourse.hw_specs import get_activation_tables
        need = {Exp, Ln, Relu, Copy}
        for set_id, funcs in enumerate(get_activation_tables(nc.m.arch).values()):
            if need <= funcs:
                nc.scalar.add_instruction(mybir.InstLoadActFuncSet(
                    name=nc.get_next_instruction_name(),
                    act_func_set_id=set_id, ins=[], outs=[]))
                break

        for _rep in range(repeat):
            c16 = const.tile([P, W16], bf, name="c16", tag="c16")

            def C16(name, rows=None):
                o, w = L16[name]
                return c16[:, o:o + w] if rows is None else c16[rows, o:o + w]

            # ---- input DMAs (sync queue): pa0(+w8), pb0, pa1, pb1, c16
            pt = {}
            for key, nm, cols in ((("a", 0), "pa0", ACOLS + W8),
                                  (("b", 0), "pb0", BCOLS),
                                  (("a", 1), "pa1", ACOLS),
                                  (("b", 1), "pb1", BCOLS)):
                t = ppool.tile([P, cols], f8, name=nm, tag=nm)
                nc.sync.dma_start(t[:], piece_d[key][:, :])
                pt[key] = t
            nc.sync.dma_start(c16[:], c16_d[:, :])

            def W8ap(name):
                o, w = L8[name]
                return pt[("a", 0)][:, o:o + w]

            def pieceA(s, off, w):
                base = W8 if s == 0 else 0
                return pt[("a", s)][:, base + off:base + off + w]

            def pieceB(s, off, w):
                return pt[("b", s)][:, off:off + w]

            def xt_pair(s, p, f):
                """xt [P, 2, P] AP for pair p, feature chunk f."""
                if p < PA:
                    ap = pieceA(s, A_OH + p * XPW + f * 2 * P, 2 * P)
                else:
                    ap = pieceB(s, B_OH + (p - PA) * XPW + f * 2 * P, 2 * P)
                return ap.rearrange("p (a b) -> p a b", a=2)

            def xt_single(s, t, f):
                """xt [P, P] AP for tile t, feature chunk f."""
                p, a = t // 2, t % 2
                if p < PA:
                    return pieceA(s, A_OH + p * XPW + f * 2 * P + a * P, P)
                return pieceB(s, B_OH + (p - PA) * XPW + f * 2 * P + a * P, P)

            # ---- rT: relu(xrootT) * croot  (Pool engine, SBUF only)
            rT = []
            for f in range(nF):
                tmp = spool.tile([P, G_cap], bf, name=f"rtmp{f}", tag=f"rtmp{f}")
                nc.gpsimd.tensor_scalar(out=tmp[:], in0=C16(f"xrootT{f}"),
                                        scalar1=0.0, scalar2=None, op0=maxop)
                t = spool.tile([P, G_cap], f8, name=f"rT{f}", tag=f"rT{f}")
                nc.gpsimd.tensor_tensor(out=t[:], in0=tmp[:], in1=C16("croot"),
                                        op=mul)
                rT.append(t)

            # ---- per (chunk, dir): stage-1 aggT, copy, l1, relu -> cbt
            cbt = [spool.tile([P, 2 * H], bf, name=f"cbt{s}", tag=f"cbt{s}")
                   for s in range(NS)]
            DI = {"bu": 0, "td": 1}
            CPY = {"td": "act", "bu": "dve"}

            def _copy(eng, dst, src_ap):
                if eng == "act":
                    nc.scalar.activation(dst, src_ap, Copy)
                else:
                    nc.vector.tensor_scalar(out=dst, in0=src_ap, scalar1=0.0,
                                            scalar2=None, op0=addop)

            def _relu_scale(eng, dst, src_ap, scale):
                if eng == "act":
                    nc.scalar.activation(dst, src_ap, Relu, scale=scale)
                else:
                    nc.vector.tensor_scalar(out=dst, in0=src_ap,
                                            scalar1=scale, scalar2=0.0,
                                            op0=mul, op1=maxop)

            aggT_ps = {}
            l1_ps = {}

            def stage1(s, d):
                # aggT psum [p, a, tgt]: f = a*P + p (DR pair layout for l1)
                ps = psA.tile([P, 2, P], f32, name=f"agg{d}{s}", tag="agg")
                aggT_ps[(s, d)] = ps
                if d == "td":
                    pairs = [(j, pieceA(s, j * 2 * P, 2 * P))
                             for j in range(A_td // 2)]
                    odd = A_td % 2
                else:
                    pairs = [(U // 2 + j, pieceB(s, j * 2 * P, 2 * P))
                             for j in range((V + W) // 2)]
                    odd = 0
                for i, (p, ohap) in enumerate(pairs):
                    oh = ohap.rearrange("p (a b) -> p a b", a=2)
                    for f in range(nF):
                        nc.tensor.matmul(out=ps[:, f, :],
                                         lhsT=xt_pair(s, p, f), rhs=oh,
                                         start=(i == 0),
                                         stop=(i == len(pairs) - 1 and not odd),
                                         perf_mode=DR)
                if odd:
                    ohap = pieceA(s, (A_td // 2) * 2 * P, P)
                    for f in range(nF):
                        nc.tensor.matmul(out=ps[:, f, :],
                                         lhsT=xt_single(s, A_td - 1, f),
                                         rhs=ohap,
                                         start=(len(pairs) == 0), stop=True)

            def l1_chain(s, d):
                sb = spool.tile([P, 2, P], f8, name=f"aT{d}{s}",
                                tag=f"aT{d}{s}")
                _copy(CPY[d], sb[:], aggT_ps[(s, d)][:])
                h = psB.tile([P, H], f32, name="hps", tag="psb")
                l1_ps[(s, d)] = h
                nc.tensor.matmul(out=h[:], lhsT=sb[:],
                                 rhs=W8ap(f"w1{d}")
                                 .rearrange("p (a b) -> p a b", a=2),
                                 start=True, stop=bool(bz), perf_mode=DR)
                if not bz:
                    nc.tensor.matmul(out=h[:],
                                     lhsT=C16("ones", rows=slice(0, 1)),
                                     rhs=C16(f"b1{d}", rows=slice(0, 1)),
                                     start=False, stop=True)

            def l1_relu(s, d):
                di = DI[d]
                _relu_scale("dve" if CPY[d] == "act" else "act",
                            cbt[s][:, di * H:(di + 1) * H],
                            l1_ps[(s, d)][:], 1.0 / (SC_OH * SC_W))

            # ---- o2 psum [P, 2, G_cap]
            o2_ps = psO.tile([P, 2, G_cap], f32, name="o2ps", tag="o2ps")

            def o2_acc(s, m_):
                nc.tensor.matmul(out=o2_ps[:, m_, :],
                                 lhsT=cbt[s][:, m_ * P:(m_ + 1) * P],
                                 rhs=C16(f"pr{s}"), start=(s == 0),
                                 stop=(s == NS - 1))

            for d in ("td", "bu"):
                stage1(0, d)
            for d in ("td", "bu"):
                l1_chain(0, d)
                l1_relu(0, d)
            for d in ("td", "bu"):
                stage1(1, d)
            for d in ("td", "bu"):
                l1_chain(1, d)
                l1_relu(1, d)
            for m_ in range(2):
                for s in range(NS):
                    o2_acc(s, m_)

            # single psum->sbuf copy for both o2 halves (DVE)
            o2_sb = spool.tile([P, 2, G_cap], f8, name="o2sb", tag="o2sb")
            _copy("dve", o2_sb[:], o2_ps[:])

            # ---- tot[d] = relu((W2s_d^T [rT; o2_d]) / SC_W + b2_d)
            tot_ps = psO.tile([P, 2, G_cap], f32, name="totps", tag="totps")
            for di, d in enumerate(("bu", "td")):
                for j in range(nW2):
                    rhs_t = rT[j][:] if j < nF else o2_sb[:, di, :]
                    nc.tensor.matmul(out=tot_ps[:, di, :],
                                     lhsT=W8ap(f"w2{d}{j}"), rhs=rhs_t,
                                     start=(j == 0), stop=(j == nW2 - 1))
            tot_sb = spool.tile([P, 2, G_cap], bf, name="totsb", tag="totsb")
            if bz:
                nc.scalar.activation(tot_sb[:], tot_ps[:], Relu,
                                     scale=1.0 / SC_W)
            else:
                for di, d in enumerate(("bu", "td")):
                    nc.scalar.activation(tot_sb[:, di, :], tot_ps[:, di, :],
                                         Relu, scale=1.0 / SC_W,
                                         bias=C16(f"b2{d}"))

            # ---- logits + log_softmax
            lg = psO.tile([G_cap, C], f32, name="lgps", tag="lgps")
            for di in range(2):
                nc.tensor.matmul(out=lg[:], lhsT=tot_sb[:, di, :G_cap],
                                 rhs=C16(f"wl{di}"), start=(di == 0),
                                 stop=(bz and di == 1))
            if not bz:
                nc.tensor.matmul(out=lg[:],
                                 lhsT=C16("ones", rows=slice(0, 1))[:, :G_cap],
                                 rhs=C16("bl", rows=slice(0, 1)),
                                 start=False, stop=True)
            ez = spool.tile([G_cap, C], f32, name="ez", tag="ez")
            se = spool.tile([G_cap, 1], f32, name="se", tag="se")
            nc.scalar.activation(ez[:], lg[:], Exp, accum_out=se[:])
            lse = spool.tile([G_cap, 1], f32, name="lse", tag="lse")
            nc.scalar.activation(lse[:], se[:], Ln)
            res = spool.tile([G_cap, C], f32, name="res", tag="res")
            nc.vector.tensor_scalar(out=res[:], in0=lg[:], scalar1=lse[:],
                                    scalar2=None, op0=sub)
            # out DMA from the Act queue (res lands right after Act's ln, so
            # the wait barely blocks it) -- keeps the SP queue a pure input
            # stream so the next repetition's input DMAs issue while this rep
            # computes
            nc.scalar.dma_start(out_d[:], res[:])

    nc.compile()
    return nc


_PROG_CACHE = {}


def _prepare_maps(inputs):
    in_maps, meta = _preprocess(inputs["x"], inputs["edge_index"],
                                inputs["batch"], inputs["num_graphs"])
    _pack_consts(in_maps, inputs, meta)
    return in_maps, meta


def _prog_key(meta):
    return (meta["F"], meta["H"], meta["C"], meta["U"], meta["V"], meta["W"],
            meta["G_cap"], meta["bz"])


def _prepare(inputs):
    in_maps, meta = _prepare_maps(inputs)
    key = _prog_key(meta)
    if key not in _PROG_CACHE:
        _PROG_CACHE[key] = _build_program(*key)
    return _PROG_CACHE[key], in_maps, meta


def kernel(**inputs):
    from concourse.bass_utils import run_bass_kernel_spmd

    nc, in_maps, meta = _prepare(inputs)
    res = run_bass_kernel_spmd(nc, in_maps, list(range(NCORES)))
    G = meta["G"]
    cog, gl = meta["core_of_graph"], meta["glocal"]
    out = np.empty((G, meta["C"]), np.float32)
    for g in range(G):
        out[g] = res.results[cog[g]]["out"][gl[g]]
    return out


# revision 19
# speedup vs baseline: 2.5791x; 2.5791x over previous
"""BiGCN (nn_BiGCN_52716428591487) Trainium2 kernel, v3.

Math: the model's output is log_softmax(cat(l2_bu[root], l2_td[root]) @ W_lin + b).
Only the layer-2 GCN values AT THE ROOT NODES matter, and GCNConv is linear in
its input features, so the whole network collapses to:

  agg1_d[v]  = sum_{e -> v} coef_d(e) * x[nbr(e)]            (v in S; self-loops
               folded into the edge list with coef dinv_d[v]^2)
  l1_d[v]    = agg1_d[v] @ W_d1 + b_d1
  out2_l1[g] = sum_{s in S_g} Pr[s, g] * relu(l1_d[s])       (layer-2 agg)
  out2_R[g]  = c_g * relu(x[root_g])                          (root-feature block
               collapses: Pr is block-diagonal by graph, c_g = sum_s Pr[s, g])
  pb/pt[g]   = relu(W2_d^T [out2_R; out2_l1_d] + b_2)
  out[g]     = log_softmax([pb, pt][g] @ W_lin + b_lin)

where S = {sources of root-incident edges} + {roots} (~1.7k of 50k nodes).

Host does index-only preprocessing (degrees, edge selection, dedup/gather
tables, the one-hot scatter matrices, Pr) plus dtype casts/layout; the device
does every arithmetic op that touches x or the weights.

Device shape: per (chunk, dir), stage-1 computes aggT = xt^T @ onehot
directly in PSUM (lhsT = gathered x rows, rhs = one-hot with coefs), so no
transposes are needed; gathered rows are deduplicated across BOTH directions
per chunk ([td-only | shared | bu-only] tile layout, one fp8 copy of x rows
serving both one-hots); l1 is a single fp8 DoubleRow matmul; the head is a
short chain of tiny matmuls + activations on [G_cap]-wide data.
W1/W2 ship as fp8 (x16 host cast-scale, descaled inside the relu
activations); one-hot coefs are x8.  5 input DMAs per core.

Sharding: graph-data parallel over 8 cores; the host concatenates the
per-core [G_cap, C] outputs.
"""

import numpy as np

P = 128
NCORES = 8
NS = 2           # slot chunks of 128 target slots per core
SC_OH = 8.0      # host scale on one-hot coefs
SC_W = 16.0      # host scale on fp8 W1/W2


def _roundup(a, m):
    return -(-int(a) // m) * m


# ----------------------------------------------------------------------------
# Host preprocessing: index-only work + gather/pack tables
# ----------------------------------------------------------------------------

def _preprocess(x, edge_index, batch, num_graphs):
    import concourse.mybir as mybir

    f8 = mybir.dt.np(mybir.dt.float8e4)

    x = np.ascontiguousarray(np.asarray(x), dtype=np.float32)
    ei = np.asarray(edge_index)
    batch = np.asarray(batch).astype(np.int64)
    G = int(np.asarray(num_graphs))
    N, F = x.shape
    nF = F // P
    src = ei[0].astype(np.int64)
    dst = ei[1].astype(np.int64)

    assert np.all(np.diff(batch) >= 0), "batch must be sorted (contiguous graphs)"
    roots = np.searchsorted(batch, np.arange(G, dtype=np.int64))  # segment_min

    deg_td = 1.0 + np.bincount(dst, minlength=N).astype(np.float64)
    deg_bu = 1.0 + np.bincount(src, minlength=N).astype(np.float64)
    dinv_td = (1.0 / np.sqrt(deg_td)).astype(np.float32)
    dinv_bu = (1.0 / np.sqrt(deg_bu)).astype(np.float32)

    G_cap = max(-(-G // NCORES), 1)

    # S: sources of root-incident edges + roots
    is_root = np.zeros(N, bool)
    is_root[roots] = True
    rmask = is_root[dst]
    r_src, r_dst = src[rmask], dst[rmask]
    r_coef = dinv_td[r_src] * dinv_td[r_dst]

    s_nodes = np.unique(np.concatenate([r_src, roots]))  # sorted
    s_graph = batch[s_nodes]
    s_count_g = np.bincount(s_graph, minlength=G)
    assert s_count_g.max() <= P, "graph S-set exceeds one chunk"

    in_s = np.zeros(N, bool)
    in_s[s_nodes] = True

    def _dir_edges(tgt_nodes, row_nodes, dinv):
        m = in_s[tgt_nodes]
        tg, rw = tgt_nodes[m], row_nodes[m]
        cf = (dinv[rw] * dinv[tg]).astype(np.float32)
        tg = np.concatenate([tg, s_nodes])          # self loops
        rw = np.concatenate([rw, s_nodes])
        cf = np.concatenate([cf, (dinv[s_nodes] ** 2).astype(np.float32)])
        return tg, rw, cf, batch[tg]

    E_td = _dir_edges(dst, src, dinv_td)
    E_bu = _dir_edges(src, dst, dinv_bu)

    # per-graph unique-row category counts (td-only / shared / bu-only)
    cat_g = np.zeros((G, 3), np.int64)
    uniq_rows_td = [None] * G
    uniq_rows_bu = [None] * G

    def _per_graph_rows(E):
        tg, rw, cf, eg = E
        order = np.argsort(eg, kind="stable")
        egs, rws = eg[order], rw[order]
        bnd = np.searchsorted(egs, np.arange(G + 1))
        return [np.unique(rws[bnd[g]:bnd[g + 1]]) for g in range(G)]

    uniq_rows_td = _per_graph_rows(E_td)
    uniq_rows_bu = _per_graph_rows(E_bu)
    for g in range(G):
        sh = np.intersect1d(uniq_rows_td[g], uniq_rows_bu[g],
                            assume_unique=True)
        cat_g[g, 1] = len(sh)
        cat_g[g, 0] = len(uniq_rows_td[g]) - len(sh)
        cat_g[g, 2] = len(uniq_rows_bu[g]) - len(sh)

    gw = cat_g.sum(axis=1)

    # graph -> (core, chunk) bin: direct greedy over NCORES*NS bins on
    # per-category loads, then local-search refinement minimizing the global
    # per-category maxima (which set the padded tile counts U/V/W).
    NB = NCORES * NS
    bin_of_graph = np.empty(G, np.int64)
    bcount = np.zeros(NB, np.int64)       # graphs per bin (core cap G_cap)
    bfill = np.zeros(NB, np.int64)        # slots per bin (cap P)
    bld = np.zeros((NB, 3))
    ccount = np.zeros(NCORES, np.int64)

    def _core_ok(b, extra=1):
        return ccount[b // NS] + extra <= G_cap * 1  # per-core graph cap

    for g in np.argsort(-gw, kind="stable"):
        cands = [b for b in range(NB)
                 if bfill[b] + s_count_g[g] <= P and ccount[b // NS] < G_cap]
        b = min(cands, key=lambda bb: ((bld[bb] + cat_g[g]).max(),
                                       bld[bb].sum()))
        bin_of_graph[g] = b
        bcount[b] += 1
        bfill[b] += s_count_g[g]
        bld[b] += cat_g[g]
        ccount[b // NS] += 1

    def _cost(loads):
        mx = loads.max(axis=0)
        u, v, w = (int(-(-m // P)) for m in mx)
        u0 = u & ~1
        t, a = u + v + w, u + v
        return (2 * t + a + (t - u0), mx.sum())

    # local search: single moves + pairwise swaps
    for _sweep in range(6):
        improved = False
        order = np.argsort(-(bld.max(axis=1)))
        for g in range(G):
            b0 = bin_of_graph[g]
            for b1 in range(NB):
                if b1 == b0:
                    continue
                if bfill[b1] + s_count_g[g] <= P and \
                        (b1 // NS == b0 // NS or ccount[b1 // NS] < G_cap):
                    new = bld.copy()
                    new[b0] -= cat_g[g]
                    new[b1] += cat_g[g]
                    if _cost(new) < _cost(bld):
                        bld = new
                        bfill[b0] -= s_count_g[g]
                        bfill[b1] += s_count_g[g]
                        ccount[b0 // NS] -= 1
                        ccount[b1 // NS] += 1
                        bcount[b0] -= 1
                        bcount[b1] += 1
                        bin_of_graph[g] = b1
                        b0 = b1
                        improved = True
        # pairwise swaps
        for g1 in range(G):
            for g2 in range(g1 + 1, G):
                b1, b2 = bin_of_graph[g1], bin_of_graph[g2]
                if b1 == b2:
                    continue
                if bfill[b1] - s_count_g[g1] + s_count_g[g2] > P or \
                        bfill[b2] - s_count_g[g2] + s_count_g[g1] > P:
                    continue
                new = bld.copy()
                new[b1] += cat_g[g2] - cat_g[g1]
                new[b2] += cat_g[g1] - cat_g[g2]
                if _cost(new) < _cost(bld):
                    bld = new
                    bfill[b1] += s_count_g[g2] - s_count_g[g1]
                    bfill[b2] += s_count_g[g1] - s_count_g[g2]
                    bin_of_graph[g1], bin_of_graph[g2] = b2, b1
                    improved = True
        if not improved:
            break

    core_of_graph = bin_of_graph // NS
    chunk_of_graph = bin_of_graph % NS
    glocal = np.empty(G, np.int64)
    counts = np.zeros(NCORES, np.int64)
    for g in range(G):
        glocal[g] = counts[core_of_graph[g]]
        counts[core_of_graph[g]] += 1

    # slot layout per bin
    s_lookup = np.full(N, -1, np.int64)   # node -> core slot (chunk*P + i)
    binfill = np.zeros(NB, np.int64)
    for g in range(G):
        b = bin_of_graph[g]
        s = chunk_of_graph[g]
        idx = s_nodes[s_graph == g]
        s_lookup[idx] = s * P + binfill[b] + np.arange(len(idx))
        binfill[b] += s_count_g[g]
    assert (binfill <= P).all()

    # ---- per (core, chunk): union rows in [td_only | shared | bu_only]
    # category layout; per dir entry lists reference row positions
    rows_cat = {}   # (c, s) -> (rows_to, rows_sh, rows_bo)
    for c in range(NCORES):
        for s in range(NS):
            gs = np.flatnonzero((core_of_graph == c) & (chunk_of_graph == s))
            rt = (np.concatenate([uniq_rows_td[g] for g in gs])
                  if len(gs) else np.empty(0, np.int64))
            rb = (np.concatenate([uniq_rows_bu[g] for g in gs])
                  if len(gs) else np.empty(0, np.int64))
            sh = np.intersect1d(rt, rb, assume_unique=True)
            to = np.setdiff1d(rt, sh, assume_unique=True)
            bo = np.setdiff1d(rb, sh, assume_unique=True)
            rows_cat[(c, s)] = (to, sh, bo)

    # uniform category tile counts (no parity padding; bu's pair range
    # starts at the even floor U0 and both dirs may end with an odd single)
    U = _roundup(max(len(rows_cat[k][0]) for k in rows_cat), P) // P
    V = _roundup(max(len(rows_cat[k][1]) for k in rows_cat), P) // P
    W = _roundup(max(len(rows_cat[k][2]) for k in rows_cat), P) // P
    T = U + V + W                                  # xt tiles per chunk
    A_td = U + V                                   # td tile range [0, A_td)
    PA = -(-A_td // 2)                             # xt pairs in piece A

    # entry maps per (c, s, d): (row_pos_in_chunk_layout, tgt_local, coef)
    ents = {}
    for d, E in (("td", E_td), ("bu", E_bu)):
        tg, rw, cf, eg = E
        slot = s_lookup[tg]
        assert (slot >= 0).all()
        core = core_of_graph[eg]
        chunk = slot // P
        for c in range(NCORES):
            for s in range(NS):
                m = (core == c) & (chunk == s)
                rw_m, cf_m = rw[m], cf[m]
                tloc = (slot[m] - s * P).astype(np.int64)
                to, sh, bo = rows_cat[(c, s)]
                # map node id -> row position in the chunk layout
                pos = np.full(len(rw_m), -1, np.int64)
                for base, cat in ((0, to), (U * P, sh), ((U + V) * P, bo)):
                    idx = np.searchsorted(cat, rw_m)
                    idx_c = np.clip(idx, 0, max(len(cat) - 1, 0))
                    hit = (len(cat) > 0) & (cat[idx_c] == rw_m) if len(cat) \
                        else np.zeros(len(rw_m), bool)
                    pos = np.where(hit, base + idx_c, pos)
                assert (pos >= 0).all()
                ents[(c, s, d)] = (pos, tloc, cf_m)

    # layer-2 aggregation matrix Pr[core, slot, glocal] and root tables
    r_graph = batch[r_dst]
    S_cap = NS * P
    Pr = np.zeros((NCORES, S_cap, G_cap), np.float32)
    np.add.at(Pr, (core_of_graph[r_graph], s_lookup[r_src], glocal[r_graph]),
              r_coef)
    np.add.at(Pr, (core_of_graph[np.arange(G)], s_lookup[roots], glocal),
              dinv_td[roots] ** 2)

    # ---- pack per-core pieces
    # piece A (per chunk): oh_td blocks + xt pairs [0, PA)
    # piece B: oh_bu blocks + xt pairs [PA, T/2)
    U0 = U & ~1
    NXP = T // 2

    def _pack(c, s):
        to, sh, bo = rows_cat[(c, s)]
        # x rows in chunk layout [T*P, F] fp8
        xg = np.zeros((T * P, F), f8)
        xg[:len(to)] = x[to].astype(f8)
        xg[U * P:U * P + len(sh)] = x[sh].astype(f8)
        xg[(U + V) * P:(U + V) * P + len(bo)] = x[bo].astype(f8)
        xg = xg.reshape(T, P, F)
        # xt pair blocks [P, nF, 2, P] -> [P, nF*2*P] per pair
        xp = xg[:2 * NXP].reshape(NXP, 2, P, nF, P).transpose(2, 0, 3, 1, 4) \
            .reshape(P, NXP, nF * 2 * P)

        def _oh(d, tile_lo, n_tiles):
            posm, tloc, cf = ents[(c, s, d)]
            oh = np.zeros((n_tiles, P, P), np.float32)
            t_idx = posm // P - tile_lo
            assert (t_idx >= 0).all() and (t_idx < n_tiles).all()
            np.add.at(oh, (t_idx, posm % P, tloc), cf * SC_OH)
            return oh.astype(f8)

        def _blocks(oh, n_tiles):
            out = [np.stack([oh[2 * j], oh[2 * j + 1]], axis=1)
                   .reshape(P, 2 * P) for j in range(n_tiles // 2)]
            if n_tiles % 2:
                out.append(oh[n_tiles - 1])
            return out

        td_blocks = _blocks(_oh("td", 0, A_td), A_td)
        bu_blocks = _blocks(_oh("bu", U0, T - U0), T - U0)
        a_parts = td_blocks + [xp[:, :PA].reshape(P, -1)]
        b_parts = bu_blocks + [xp[:, PA:].reshape(P, -1)]
        if T % 2:
            # last lone tile stored as a single block [P, nF*P] in piece B
            xs = np.ascontiguousarray(xg[T - 1].reshape(P, nF, P)
                                      .reshape(P, nF * P))
            b_parts.append(xs)
        pieceA = np.concatenate(a_parts, axis=1)
        pieceB = np.concatenate(b_parts, axis=1)
        return (np.ascontiguousarray(pieceA), np.ascontiguousarray(pieceB))

    in_maps = []
    for c in range(NCORES):
        m = {}
        for s in range(NS):
            a, b = _pack(c, s)
            m[f"pa{s}"] = a
            m[f"pb{s}"] = b
        m["pr"] = Pr[c]
        gs = np.flatnonzero(core_of_graph == c)
        xrootT = np.zeros((F, G_cap), np.float32)
        xrootT[:, glocal[gs]] = x[roots[gs]].T
        m["xrootT"] = xrootT
        m["croot"] = np.tile(Pr[c].sum(axis=0, dtype=np.float64)
                             .astype(np.float32), (P, 1))
        in_maps.append(m)

    meta = dict(F=F, U=U, V=V, W=W, G_cap=G_cap, counts=counts, G=G,
                core_of_graph=core_of_graph, glocal=glocal)
    return in_maps, meta


def _c16_layout(F, H, C, G_cap, bz):
    """Column layout of the bf16 constant matrix [P, W16]."""
    off = 0
    L = {}

    def add(name, w):
        nonlocal off
        L[name] = (off, w)
        off += w

    for f in range(2 * H // P):
        add(f"wl{f}", C)
    if not bz:
        add("b1td", H)
        add("b1bu", H)
        add("bl", C)
        add("ones", P)
        add("b2bu", 1)
        add("b2td", 1)
    for s in range(NS):
        add(f"pr{s}", G_cap)
    for f in range(F // P):
        add(f"xrootT{f}", G_cap)
    add("croot", G_cap)
    return L, off


def _w8_layout(F, H):
    """Column layout of the fp8 scaled-weight block (rides in piece pa0)."""
    off = 0
    L = {}

    def add(name, w):
        nonlocal off
        L[name] = (off, w)
        off += w

    add("w1td", 2 * H)            # [P, 2, H] f-interleaved pair for DR
    add("w1bu", 2 * H)
    for d in ("bu", "td"):
        for j in range((F + H) // P):
            add(f"w2{d}{j}", H)   # [P, H] chunks as lhsT
    return L, off


def _pack_consts(in_maps, inputs, meta):
    import concourse.mybir as mybir
    import ml_dtypes

    f8 = mybir.dt.np(mybir.dt.float8e4)
    bf16 = ml_dtypes.bfloat16
    H = int(np.asarray(inputs["W_td1"]).shape[1])
    C = int(np.asarray(inputs["W_lin"]).shape[1])
    F, G_cap = meta["F"], meta["G_cap"]
    bz = all(not np.any(np.asarray(inputs[k]))
             for k in ("b_td1", "b_bu1", "b_bu2", "b_td2", "b_lin"))
    L16, W16 = _c16_layout(F, H, C, G_cap, bz)
    L8, W8 = _w8_layout(F, H)
    g = lambda k: np.asarray(inputs[k], dtype=np.float32)

    base16 = np.zeros((P, W16), bf16)

    def put(name, block):
        o, w = L16[name]
        base16[:, o:o + w][tuple(slice(s) for s in block.shape)] = \
            block.astype(bf16)

    for f in range(2 * H // P):
        put(f"wl{f}", g("W_lin")[f * P:(f + 1) * P, :])
    if not bz:
        put("b1td", g("b_td1").reshape(1, H))
        put("b1bu", g("b_bu1").reshape(1, H))
        put("bl", g("b_lin").reshape(1, C))
        put("ones", np.ones((1, P), np.float32))
        put("b2bu", g("b_bu2")[:P, None])
        put("b2td", g("b_td2")[:P, None])

    w8 = np.zeros((P, W8), f8)

    def put8(name, block):
        o, w = L8[name]
        w8[:, o:o + w] = block.astype(f8)

    for d, wn in (("td", "W_td1"), ("bu", "W_bu1")):
        w1 = g(wn) * SC_W                       # [F, H] = [2P, H]
        put8(f"w1{d}", w1.reshape(2, P, H).transpose(1, 0, 2).reshape(P, 2 * H))
    for d, wn in (("bu", "W_bu2"), ("td", "W_td2")):
        w2 = g(wn) * SC_W                       # [F+H, H]
        for j in range((F + H) // P):
            put8(f"w2{d}{j}", w2[j * P:(j + 1) * P, :])

    for m in in_maps:
        c16 = base16.copy()
        pr = m.pop("pr")
        for s in range(NS):
            o, w = L16[f"pr{s}"]
            c16[:, o:o + w] = pr[s * P:(s + 1) * P].astype(bf16)
        xrootT = m.pop("xrootT")
        for f in range(F // P):
            o, w = L16[f"xrootT{f}"]
            c16[:, o:o + w] = xrootT[f * P:(f + 1) * P].astype(bf16)
        o, w = L16["croot"]
        c16[:, o:o + w] = m.pop("croot").astype(bf16)
        m["c16"] = np.ascontiguousarray(c16)
        m["pa0"] = np.ascontiguousarray(np.concatenate([w8, m["pa0"]], axis=1))

    meta["H"], meta["C"] = H, C
    meta["bz"] = bz
    return H


# ----------------------------------------------------------------------------
# Device program
# ----------------------------------------------------------------------------

def _build_program(F, H, C, U, V, W, G_cap, bz=False, repeat=1):
    from contextlib import ExitStack

    import concourse.bacc as bacc
    import concourse.bass as bass  # noqa: F401
    import concourse.mybir as mybir
    import concourse.tile as tile

    dt = mybir.dt
    f32, bf, f8 = dt.float32, dt.bfloat16, dt.float8e4
    nF = F // P
    nW2 = (F + H) // P
    assert F % P == 0 and H == P and nF == 2
    L16, W16 = _c16_layout(F, H, C, G_cap, bz)
    L8, W8 = _w8_layout(F, H)

    T = U + V + W
    U0 = U & ~1                         # bu tile range [U0, T), pair-aligned
    NB_T = T - U0                       # bu tiles
    A_td = U + V
    PA = -(-A_td // 2)
    NXP = T // 2                        # full xt pair blocks
    XPW = nF * 2 * P                    # xt pair block cols
    # piece A: oh_td ((A_td//2) pair blocks + odd single) + xt pairs [0, PA)
    A_OH = (A_td // 2) * 2 * P + (A_td % 2) * P
    ACOLS = A_OH + PA * XPW
    # piece B: oh_bu (pairs + odd single) + xt pairs [PA, NXP) + lone tile
    B_OH = (NB_T // 2) * 2 * P + (NB_T % 2) * P
    B_XS = B_OH + (NXP - PA) * XPW      # offset of the lone last xt tile
    BCOLS = B_XS + (T % 2) * nF * P

    mul, sub, addop, maxop = (
        mybir.AluOpType.mult, mybir.AluOpType.subtract,
        mybir.AluOpType.add, mybir.AluOpType.max)
    Relu, Exp, Ln, Copy = (mybir.ActivationFunctionType.Relu,
                           mybir.ActivationFunctionType.Exp,
                           mybir.ActivationFunctionType.Ln,
                           mybir.ActivationFunctionType.Copy)
    DR = mybir.MatmulPerfMode.DoubleRow

    nc = bacc.Bacc("TRN2", target_bir_lowering=False, debug=False,
                   num_devices=NCORES)

    piece_d = {}
    for s in range(NS):
        piece_d[("a", s)] = nc.dram_tensor(
            f"pa{s}", [P, ACOLS + (W8 if s == 0 else 0)], f8,
            kind="ExternalInput").ap()
        piece_d[("b", s)] = nc.dram_tensor(
            f"pb{s}", [P, BCOLS], f8, kind="ExternalInput").ap()
    c16_d = nc.dram_tensor("c16", [P, W16], bf, kind="ExternalInput").ap()
    out_d = nc.dram_tensor("out", [G_cap, C], f32, kind="ExternalOutput").ap()

    with ExitStack() as ctx:
        tc = ctx.enter_context(tile.TileContext(nc))
        const = ctx.enter_context(tc.tile_pool(
            name="cst", bufs=(1 if repeat == 1 else 2)))
        ppool = ctx.enter_context(tc.tile_pool(name="pp", bufs=2))
        spool = ctx.enter_context(tc.tile_pool(name="sp", bufs=2))
        psA = ctx.enter_context(tc.tile_pool(name="psA", bufs=2, space="PSUM"))
        psB = ctx.enter_context(tc.tile_pool(name="psB", bufs=2, space="PSUM"))
        psO = ctx.enter_context(tc.tile_pool(name="psO", bufs=1, space="PSUM"))

        # load the one act table containing Exp/Ln/Relu/Copy up-front
        from concourse.hw_specs import get_activation_tables
        need = {Exp, Ln, Relu, Copy}
        for set_id, funcs in enumerate(get_activation_tables(nc.m.arch).values()):
            if need <= funcs:
                nc.scalar.add_instruction(mybir.InstLoadActFuncSet(
                    name=nc.get_next_instruction_name(),
                    act_func_set_id=set_id, ins=[], outs=[]))
                break

        for _rep in range(repeat):
            c16 = const.tile([P, W16], bf, name="c16", tag="c16")

            def C16(name, rows=None):
                o, w = L16[name]
                return c16[:, o:o + w] if rows is None else c16[rows, o:o + w]

            # ---- input DMAs (sync queue): pa0(+w8), pb0, pa1, pb1, c16
            pt = {}
            for key, nm, cols in ((("a", 0), "pa0", ACOLS + W8),
                                  (("b", 0), "pb0", BCOLS),
                                  (("a", 1), "pa1", ACOLS)):
                t = ppool.tile([P, cols], f8, name=nm, tag=nm)
                nc.sync.dma_start(t[:], piece_d[key][:, :])
                pt[key] = t
            nc.sync.dma_start(c16[:], c16_d[:, :])
            t = ppool.tile([P, BCOLS], f8, name="pb1", tag="pb1")
            nc.sync.dma_start(t[:], piece_d[("b", 1)][:, :])
            pt[("b", 1)] = t

            def W8ap(name):
                o, w = L8[name]
                return pt[("a", 0)][:, o:o + w]

            def pieceA(s, off, w):
                base = W8 if s == 0 else 0
                return pt[("a", s)][:, base + off:base + off + w]

            def pieceB(s, off, w):
                return pt[("b", s)][:, off:off + w]

            def xt_pair(s, p, f):
                """xt [P, 2, P] AP for pair p, feature chunk f."""
                if p < PA:
                    ap = pieceA(s, A_OH + p * XPW + f * 2 * P, 2 * P)
                else:
                    ap = pieceB(s, B_OH + (p - PA) * XPW + f * 2 * P, 2 * P)
                return ap.rearrange("p (a b) -> p a b", a=2)

            def xt_single(s, t, f):
                """xt [P, P] AP for tile t, feature chunk f."""
                if t == T - 1 and T % 2:
                    return pieceB(s, B_XS + f * P, P)
                p, a = t // 2, t % 2
                if p < PA:
                    return pieceA(s, A_OH + p * XPW + f * 2 * P + a * P, P)
                return pieceB(s, B_OH + (p - PA) * XPW + f * 2 * P + a * P, P)

            # ---- rT: relu(xrootT) * croot  (Pool engine, SBUF only)
            rT = []
            for f in range(nF):
                tmp = spool.tile([P, G_cap], bf, name=f"rtmp{f}", tag=f"rtmp{f}")
                nc.gpsimd.tensor_scalar(out=tmp[:], in0=C16(f"xrootT{f}"),
                                        scalar1=0.0, scalar2=None, op0=maxop)
                t = spool.tile([P, G_cap], f8, name=f"rT{f}", tag=f"rT{f}")
                nc.gpsimd.tensor_tensor(out=t[:], in0=tmp[:], in1=C16("croot"),
                                        op=mul)
                rT.append(t)

            # ---- per (chunk, dir): stage-1 aggT, copy, l1, relu -> cbt
            cbt = [spool.tile([P, 2 * H], bf, name=f"cbt{s}", tag=f"cbt{s}")
                   for s in range(NS)]
            DI = {"bu": 0, "td": 1}
            CPY = {"td": "act", "bu": "dve"}

            def _copy(eng, dst, src_ap):
                if eng == "act":
                    nc.scalar.activation(dst, src_ap, Copy)
                else:
                    nc.vector.tensor_scalar(out=dst, in0=src_ap, scalar1=0.0,
                                            scalar2=None, op0=addop)

            def _relu_scale(eng, dst, src_ap, scale):
                if eng == "act":
                    nc.scalar.activation(dst, src_ap, Relu, scale=scale)
                else:
                    nc.vector.tensor_scalar(out=dst, in0=src_ap,
                                            scalar1=scale, scalar2=0.0,
                                            op0=mul, op1=maxop)

            aggT_ps = {}
            l1_ps = {}

            def stage1(s, d):
                # aggT psum [p, a, tgt]: f = a*P + p (DR pair layout for l1)
                ps = psA.tile([P, 2, P], f32, name=f"agg{d}{s}", tag="agg")
                aggT_ps[(s, d)] = ps
                if d == "td":
                    pairs = [(j, pieceA(s, j * 2 * P, 2 * P))
                             for j in range(A_td // 2)]
                    odd = A_td % 2
                    odd_t = A_td - 1
                    odd_oh = pieceA(s, (A_td // 2) * 2 * P, P)
                else:
                    pairs = [(U0 // 2 + j, pieceB(s, j * 2 * P, 2 * P))
                             for j in range(NB_T // 2)]
                    odd = NB_T % 2
                    odd_t = T - 1
                    odd_oh = pieceB(s, (NB_T // 2) * 2 * P, P)
                for i, (p, ohap) in enumerate(pairs):
                    oh = ohap.rearrange("p (a b) -> p a b", a=2)
                    for f in range(nF):
                        nc.tensor.matmul(out=ps[:, f, :],
                                         lhsT=xt_pair(s, p, f), rhs=oh,
                                         start=(i == 0),
                                         stop=(i == len(pairs) - 1 and not odd),
                                         perf_mode=DR)
                if odd:
                    for f in range(nF):
                        nc.tensor.matmul(out=ps[:, f, :],
                                         lhsT=xt_single(s, odd_t, f),
                                         rhs=odd_oh,
                                         start=(len(pairs) == 0), stop=True)

            def l1_chain(s, d):
                sb = spool.tile([P, 2, P], f8, name=f"aT{d}{s}",
                                tag=f"aT{d}{s}")
                _copy(CPY[d], sb[:], aggT_ps[(s, d)][:])
                h = psB.tile([P, H], f32, name="hps", tag="psb")
                l1_ps[(s, d)] = h
                nc.tensor.matmul(out=h[:], lhsT=sb[:],
                                 rhs=W8ap(f"w1{d}")
                                 .rearrange("p (a b) -> p a b", a=2),
                                 start=True, stop=bool(bz), perf_mode=DR)
                if not bz:
                    nc.tensor.matmul(out=h[:],
                                     lhsT=C16("ones", rows=slice(0, 1)),
                                     rhs=C16(f"b1{d}", rows=slice(0, 1)),
                                     start=False, stop=True)

            def l1_relu(s, d):
                di = DI[d]
                _relu_scale("dve" if CPY[d] == "act" else "act",
                            cbt[s][:, di * H:(di + 1) * H],
                            l1_ps[(s, d)][:], 1.0 / (SC_OH * SC_W))

            # ---- o2 psum [P, 2, G_cap]
            o2_ps = psO.tile([P, 2, G_cap], f32, name="o2ps", tag="o2ps")

            def o2_acc(s, m_):
                nc.tensor.matmul(out=o2_ps[:, m_, :],
                                 lhsT=cbt[s][:, m_ * P:(m_ + 1) * P],
                                 rhs=C16(f"pr{s}"), start=(s == 0),
                                 stop=(s == NS - 1))

            for d in ("td", "bu"):
                stage1(0, d)
            for d in ("td", "bu"):
                l1_chain(0, d)
                l1_relu(0, d)
            for d in ("bu", "td"):
                stage1(1, d)
            for d in ("bu", "td"):
                l1_chain(1, d)
                l1_relu(1, d)
            for m_ in range(2):
                for s in range(NS):
                    o2_acc(s, m_)

            # single psum->sbuf copy for both o2 halves (DVE)
            o2_sb = spool.tile([P, 2, G_cap], f8, name="o2sb", tag="o2sb")
            _copy("dve", o2_sb[:], o2_ps[:])

            # ---- tot[d] = relu((W2s_d^T [rT; o2_d]) / SC_W + b2_d)
            tot_ps = psO.tile([P, 2, G_cap], f32, name="totps", tag="totps")
            for di, d in enumerate(("bu", "td")):
                for j in range(nW2):
                    rhs_t = rT[j][:] if j < nF else o2_sb[:, di, :]
                    nc.tensor.matmul(out=tot_ps[:, di, :],
                                     lhsT=W8ap(f"w2{d}{j}"), rhs=rhs_t,
                                     start=(j == 0), stop=(j == nW2 - 1))
            tot_sb = spool.tile([P, 2, G_cap], bf, name="totsb", tag="totsb")
            if bz:
                nc.scalar.activation(tot_sb[:], tot_ps[:], Relu,
                                     scale=1.0 / SC_W)
            else:
                for di, d in enumerate(("bu", "td")):
                    nc.scalar.activation(tot_sb[:, di, :], tot_ps[:, di, :],
                                         Relu, scale=1.0 / SC_W,
                                         bias=C16(f"b2{d}"))

            # ---- logits + log_softmax
            lg = psO.tile([G_cap, C], f32, name="lgps", tag="lgps")
            for di in range(2):
                nc.tensor.matmul(out=lg[:], lhsT=tot_sb[:, di, :G_cap],
                                 rhs=C16(f"wl{di}"), start=(di == 0),
                                 stop=(bz and di == 1))
            if not bz:
                nc.tensor.matmul(out=lg[:],
                                 lhsT=C16("ones", rows=slice(0, 1))[:, :G_cap],
                                 rhs=C16("bl", rows=slice(0, 1)),
                                 start=False, stop=True)
            ez = spool.tile([G_cap, C], f32, name="ez", tag="ez")
            se = spool.tile([G_cap, 1], f32, name="se", tag="se")
            nc.scalar.activation(ez[:], lg[:], Exp, accum_out=se[:])
            lse = spool.tile([G_cap, 1], f32, name="lse", tag="lse")
            nc.scalar.activation(lse[:], se[:], Ln)
            res = spool.tile([G_cap, C], f32, name="res", tag="res")
            nc.vector.tensor_scalar(out=res[:], in0=lg[:], scalar1=lse[:],
                                    scalar2=None, op0=sub)
            # out DMA from the Act queue (res lands right after Act's ln, so
            # the wait barely blocks it) -- keeps the SP queue a pure input
            # stream so the next repetition's input DMAs issue while this rep
            # computes
            nc.scalar.dma_start(out_d[:], res[:])

    nc.compile()
    return nc


_PROG_CACHE = {}


def _prepare_maps(inputs):
    in_maps, meta = _preprocess(inputs["x"], inputs["edge_index"],
                                inputs["batch"], inputs["num_graphs"])
    _pack_consts(in_maps, inputs, meta)
    return in_maps, meta


def _prog_key(meta):
    return (meta["F"], meta["H"], meta["C"], meta["U"], meta["V"], meta["W"],
            meta["G_cap"], meta["bz"])


def _prepare(inputs):
    in_maps, meta = _prepare_maps(inputs)
    key = _prog_key(meta)
    if key not in _PROG_CACHE:
        _PROG_CACHE[key] = _build_program(*key)
    return _PROG_CACHE[key], in_maps, meta


def kernel(**inputs):
    from concourse.bass_utils import run_bass_kernel_spmd

    nc, in_maps, meta = _prepare(inputs)
    res = run_bass_kernel_spmd(nc, in_maps, list(range(NCORES)))
    G = meta["G"]
    cog, gl = meta["core_of_graph"], meta["glocal"]
    out = np.empty((G, meta["C"]), np.float32)
    for g in range(G):
        out[g] = res.results[cog[g]]["out"][gl[g]]
    return out


# revision 23
# speedup vs baseline: 8.2995x; 3.2180x over previous
"""BiGCN (nn_BiGCN_52716428591487) Trainium2 kernel, v3.

Math: the model's output is log_softmax(cat(l2_bu[root], l2_td[root]) @ W_lin + b).
Only the layer-2 GCN values AT THE ROOT NODES matter, and GCNConv is linear in
its input features, so the whole network collapses to:

  agg1_d[v]  = sum_{e -> v} coef_d(e) * x[nbr(e)]            (v in S; self-loops
               folded into the edge list with coef dinv_d[v]^2)
  l1_d[v]    = agg1_d[v] @ W_d1 + b_d1
  out2_l1[g] = sum_{s in S_g} Pr[s, g] * relu(l1_d[s])       (layer-2 agg)
  out2_R[g]  = c_g * relu(x[root_g])                          (root-feature block
               collapses: Pr is block-diagonal by graph, c_g = sum_s Pr[s, g])
  pb/pt[g]   = relu(W2_d^T [out2_R; out2_l1_d] + b_2)
  out[g]     = log_softmax([pb, pt][g] @ W_lin + b_lin)

where S = {sources of root-incident edges} + {roots} (~1.7k of 50k nodes).

Host does index-only preprocessing (degrees, edge selection, dedup/gather
tables, the one-hot scatter matrices, Pr) plus dtype casts/layout; the device
does every arithmetic op that touches x or the weights.

Device shape: per (chunk, dir), stage-1 computes aggT = xt^T @ onehot
directly in PSUM (lhsT = gathered x rows, rhs = one-hot with coefs), so no
transposes are needed; gathered rows are deduplicated across BOTH directions
per chunk ([td-only | shared | bu-only] tile layout, one fp8 copy of x rows
serving both one-hots); l1 is a single fp8 DoubleRow matmul; the head is a
short chain of tiny matmuls + activations on [G_cap]-wide data.
W1/W2 ship as fp8 (x16 host cast-scale, descaled inside the relu
activations); one-hot coefs are x8.  5 input DMAs per core.

Sharding: graph-data parallel over 8 cores; the host concatenates the
per-core [G_cap, C] outputs.
"""

import numpy as np

P = 128
NCORES = 8
NS = 2           # slot chunks of 128 target slots per core
SC_OH = 8.0      # host scale on one-hot coefs
SC_W = 16.0      # host scale on fp8 W1/W2


def _roundup(a, m):
    return -(-int(a) // m) * m


# ----------------------------------------------------------------------------
# Host preprocessing: index-only work + gather/pack tables
# ----------------------------------------------------------------------------

def _preprocess(x, edge_index, batch, num_graphs):
    import concourse.mybir as mybir

    f8 = mybir.dt.np(mybir.dt.float8e4)

    x = np.ascontiguousarray(np.asarray(x), dtype=np.float32)
    ei = np.asarray(edge_index)
    batch = np.asarray(batch).astype(np.int64)
    G = int(np.asarray(num_graphs))
    N, F = x.shape
    nF = F // P
    src = ei[0].astype(np.int64)
    dst = ei[1].astype(np.int64)

    assert np.all(np.diff(batch) >= 0), "batch must be sorted (contiguous graphs)"
    roots = np.searchsorted(batch, np.arange(G, dtype=np.int64))  # segment_min

    deg_td = 1.0 + np.bincount(dst, minlength=N).astype(np.float64)
    deg_bu = 1.0 + np.bincount(src, minlength=N).astype(np.float64)
    dinv_td = (1.0 / np.sqrt(deg_td)).astype(np.float32)
    dinv_bu = (1.0 / np.sqrt(deg_bu)).astype(np.float32)

    G_cap = max(-(-G // NCORES), 1)

    # S: sources of root-incident edges + roots
    is_root = np.zeros(N, bool)
    is_root[roots] = True
    rmask = is_root[dst]
    r_src, r_dst = src[rmask], dst[rmask]
    r_coef = dinv_td[r_src] * dinv_td[r_dst]

    s_nodes = np.unique(np.concatenate([r_src, roots]))  # sorted
    s_graph = batch[s_nodes]
    s_count_g = np.bincount(s_graph, minlength=G)
    assert s_count_g.max() <= P, "graph S-set exceeds one chunk"

    in_s = np.zeros(N, bool)
    in_s[s_nodes] = True

    def _dir_edges(tgt_nodes, row_nodes, dinv):
        m = in_s[tgt_nodes]
        tg, rw = tgt_nodes[m], row_nodes[m]
        cf = (dinv[rw] * dinv[tg]).astype(np.float32)
        tg = np.concatenate([tg, s_nodes])          # self loops
        rw = np.concatenate([rw, s_nodes])
        cf = np.concatenate([cf, (dinv[s_nodes] ** 2).astype(np.float32)])
        return tg, rw, cf, batch[tg]

    E_td = _dir_edges(dst, src, dinv_td)
    E_bu = _dir_edges(src, dst, dinv_bu)

    # per-graph unique-row category counts (td-only / shared / bu-only)
    cat_g = np.zeros((G, 3), np.int64)
    uniq_rows_td = [None] * G
    uniq_rows_bu = [None] * G

    def _per_graph_rows(E):
        tg, rw, cf, eg = E
        order = np.argsort(eg, kind="stable")
        egs, rws = eg[order], rw[order]
        bnd = np.searchsorted(egs, np.arange(G + 1))
        return [np.unique(rws[bnd[g]:bnd[g + 1]]) for g in range(G)]

    uniq_rows_td = _per_graph_rows(E_td)
    uniq_rows_bu = _per_graph_rows(E_bu)
    for g in range(G):
        sh = np.intersect1d(uniq_rows_td[g], uniq_rows_bu[g],
                            assume_unique=True)
        cat_g[g, 1] = len(sh)
        cat_g[g, 0] = len(uniq_rows_td[g]) - len(sh)
        cat_g[g, 2] = len(uniq_rows_bu[g]) - len(sh)

    gw = cat_g.sum(axis=1)

    # graph -> (core, chunk) bin: direct greedy over NCORES*NS bins on
    # per-category loads, then local-search refinement minimizing the global
    # per-category maxima (which set the padded tile counts U/V/W).
    NB = NCORES * NS
    bin_of_graph = np.empty(G, np.int64)
    bcount = np.zeros(NB, np.int64)       # graphs per bin (core cap G_cap)
    bfill = np.zeros(NB, np.int64)        # slots per bin (cap P)
    bld = np.zeros((NB, 3))
    ccount = np.zeros(NCORES, np.int64)

    def _core_ok(b, extra=1):
        return ccount[b // NS] + extra <= G_cap * 1  # per-core graph cap

    for g in np.argsort(-gw, kind="stable"):
        cands = [b for b in range(NB)
                 if bfill[b] + s_count_g[g] <= P and ccount[b // NS] < G_cap]
        b = min(cands, key=lambda bb: ((bld[bb] + cat_g[g]).max(),
                                       bld[bb].sum()))
        bin_of_graph[g] = b
        bcount[b] += 1
        bfill[b] += s_count_g[g]
        bld[b] += cat_g[g]
        ccount[b // NS] += 1

    def _cost(loads):
        mx = loads.max(axis=0)
        u, v, w = (int(-(-m // P)) for m in mx)
        u0 = u & ~1
        t, a = u + v + w, u + v
        return (2 * t + a + (t - u0), mx.sum())

    # local search: single moves + pairwise swaps
    for _sweep in range(6):
        improved = False
        order = np.argsort(-(bld.max(axis=1)))
        for g in range(G):
            b0 = bin_of_graph[g]
            for b1 in range(NB):
                if b1 == b0:
                    continue
                if bfill[b1] + s_count_g[g] <= P and \
                        (b1 // NS == b0 // NS or ccount[b1 // NS] < G_cap):
                    new = bld.copy()
                    new[b0] -= cat_g[g]
                    new[b1] += cat_g[g]
                    if _cost(new) < _cost(bld):
                        bld = new
                        bfill[b0] -= s_count_g[g]
                        bfill[b1] += s_count_g[g]
                        ccount[b0 // NS] -= 1
                        ccount[b1 // NS] += 1
                        bcount[b0] -= 1
                        bcount[b1] += 1
                        bin_of_graph[g] = b1
                        b0 = b1
                        improved = True
        # pairwise swaps
        for g1 in range(G):
            for g2 in range(g1 + 1, G):
                b1, b2 = bin_of_graph[g1], bin_of_graph[g2]
                if b1 == b2:
                    continue
                if bfill[b1] - s_count_g[g1] + s_count_g[g2] > P or \
                        bfill[b2] - s_count_g[g2] + s_count_g[g1] > P:
                    continue
                new = bld.copy()
                new[b1] += cat_g[g2] - cat_g[g1]
                new[b2] += cat_g[g1] - cat_g[g2]
                if _cost(new) < _cost(bld):
                    bld = new
                    bfill[b1] += s_count_g[g2] - s_count_g[g1]
                    bfill[b2] += s_count_g[g1] - s_count_g[g2]
                    bin_of_graph[g1], bin_of_graph[g2] = b2, b1
                    improved = True
        if not improved:
            break

    core_of_graph = bin_of_graph // NS
    chunk_of_graph = bin_of_graph % NS
    glocal = np.empty(G, np.int64)
    counts = np.zeros(NCORES, np.int64)
    for g in range(G):
        glocal[g] = counts[core_of_graph[g]]
        counts[core_of_graph[g]] += 1

    # slot layout per bin
    s_lookup = np.full(N, -1, np.int64)   # node -> core slot (chunk*P + i)
    binfill = np.zeros(NB, np.int64)
    for g in range(G):
        b = bin_of_graph[g]
        s = chunk_of_graph[g]
        idx = s_nodes[s_graph == g]
        s_lookup[idx] = s * P + binfill[b] + np.arange(len(idx))
        binfill[b] += s_count_g[g]
    assert (binfill <= P).all()

    # ---- per (core, chunk): union rows in [td_only | shared | bu_only]
    # category layout; per dir entry lists reference row positions
    rows_cat = {}   # (c, s) -> (rows_to, rows_sh, rows_bo)
    for c in range(NCORES):
        for s in range(NS):
            gs = np.flatnonzero((core_of_graph == c) & (chunk_of_graph == s))
            rt = (np.concatenate([uniq_rows_td[g] for g in gs])
                  if len(gs) else np.empty(0, np.int64))
            rb = (np.concatenate([uniq_rows_bu[g] for g in gs])
                  if len(gs) else np.empty(0, np.int64))
            sh = np.intersect1d(rt, rb, assume_unique=True)
            to = np.setdiff1d(rt, sh, assume_unique=True)
            bo = np.setdiff1d(rb, sh, assume_unique=True)
            rows_cat[(c, s)] = (to, sh, bo)

    # uniform category tile counts (no parity padding; bu's pair range
    # starts at the even floor U0 and both dirs may end with an odd single)
    U = _roundup(max(len(rows_cat[k][0]) for k in rows_cat), P) // P
    V = _roundup(max(len(rows_cat[k][1]) for k in rows_cat), P) // P
    W = _roundup(max(len(rows_cat[k][2]) for k in rows_cat), P) // P
    T = U + V + W                                  # xt tiles per chunk
    A_td = U + V                                   # td tile range [0, A_td)
    PA = -(-A_td // 2)                             # xt pairs in piece A

    # entry maps per (c, s, d): (row_pos_in_chunk_layout, tgt_local, coef)
    ents = {}
    for d, E in (("td", E_td), ("bu", E_bu)):
        tg, rw, cf, eg = E
        slot = s_lookup[tg]
        assert (slot >= 0).all()
        core = core_of_graph[eg]
        chunk = slot // P
        for c in range(NCORES):
            for s in range(NS):
                m = (core == c) & (chunk == s)
                rw_m, cf_m = rw[m], cf[m]
                tloc = (slot[m] - s * P).astype(np.int64)
                to, sh, bo = rows_cat[(c, s)]
                # map node id -> row position in the chunk layout
                pos = np.full(len(rw_m), -1, np.int64)
                for base, cat in ((0, to), (U * P, sh), ((U + V) * P, bo)):
                    idx = np.searchsorted(cat, rw_m)
                    idx_c = np.clip(idx, 0, max(len(cat) - 1, 0))
                    hit = (len(cat) > 0) & (cat[idx_c] == rw_m) if len(cat) \
                        else np.zeros(len(rw_m), bool)
                    pos = np.where(hit, base + idx_c, pos)
                assert (pos >= 0).all()
                ents[(c, s, d)] = (pos, tloc, cf_m)

    # layer-2 aggregation matrix Pr[core, slot, glocal] and root tables
    r_graph = batch[r_dst]
    S_cap = NS * P
    Pr = np.zeros((NCORES, S_cap, G_cap), np.float32)
    np.add.at(Pr, (core_of_graph[r_graph], s_lookup[r_src], glocal[r_graph]),
              r_coef)
    np.add.at(Pr, (core_of_graph[np.arange(G)], s_lookup[roots], glocal),
              dinv_td[roots] ** 2)

    # ---- pack per-core pieces
    # piece A (per chunk): oh_td blocks + xt pairs [0, PA)
    # piece B: oh_bu blocks + xt pairs [PA, T/2)
    U0 = U & ~1
    NXP = T // 2

    def _pack(c, s):
        to, sh, bo = rows_cat[(c, s)]
        # x rows in chunk layout [T*P, F] fp8
        xg = np.zeros((T * P, F), f8)
        xg[:len(to)] = x[to].astype(f8)
        xg[U * P:U * P + len(sh)] = x[sh].astype(f8)
        xg[(U + V) * P:(U + V) * P + len(bo)] = x[bo].astype(f8)
        xg = xg.reshape(T, P, F)
        # xt pair blocks [P, nF, 2, P] -> [P, nF*2*P] per pair
        xp = xg[:2 * NXP].reshape(NXP, 2, P, nF, P).transpose(2, 0, 3, 1, 4) \
            .reshape(P, NXP, nF * 2 * P)

        def _oh(d, tile_lo, n_tiles):
            posm, tloc, cf = ents[(c, s, d)]
            oh = np.zeros((n_tiles, P, P), np.float32)
            t_idx = posm // P - tile_lo
            assert (t_idx >= 0).all() and (t_idx < n_tiles).all()
            np.add.at(oh, (t_idx, posm % P, tloc), cf * SC_OH)
            return oh.astype(f8)

        def _blocks(oh, n_tiles):
            out = [np.stack([oh[2 * j], oh[2 * j + 1]], axis=1)
                   .reshape(P, 2 * P) for j in range(n_tiles // 2)]
            if n_tiles % 2:
                out.append(oh[n_tiles - 1])
            return out

        td_blocks = _blocks(_oh("td", 0, A_td), A_td)
        bu_blocks = _blocks(_oh("bu", U0, T - U0), T - U0)
        a_parts = td_blocks + [xp[:, :PA].reshape(P, -1)]
        b_parts = bu_blocks + [xp[:, PA:].reshape(P, -1)]
        if T % 2:
            # last lone tile stored as a single block [P, nF*P] in piece B
            xs = np.ascontiguousarray(xg[T - 1].reshape(P, nF, P)
                                      .reshape(P, nF * P))
            b_parts.append(xs)
        pieceA = np.concatenate(a_parts, axis=1)
        pieceB = np.concatenate(b_parts, axis=1)
        return (np.ascontiguousarray(pieceA), np.ascontiguousarray(pieceB))

    in_maps = []
    for c in range(NCORES):
        m = {}
        for s in range(NS):
            a, b = _pack(c, s)
            m[f"pa{s}"] = a
            m[f"pb{s}"] = b
        m["pr"] = Pr[c]
        gs = np.flatnonzero(core_of_graph == c)
        xrootT = np.zeros((F, G_cap), np.float32)
        xrootT[:, glocal[gs]] = x[roots[gs]].T
        m["xrootT"] = xrootT
        m["croot"] = np.tile(Pr[c].sum(axis=0, dtype=np.float64)
                             .astype(np.float32), (P, 1))
        in_maps.append(m)

    meta = dict(F=F, U=U, V=V, W=W, G_cap=G_cap, counts=counts, G=G,
                core_of_graph=core_of_graph, glocal=glocal)
    return in_maps, meta


def _c16_layout(F, H, C, G_cap, bz):
    """Column layout of the bf16 constant matrix [P, W16]."""
    off = 0
    L = {}

    def add(name, w):
        nonlocal off
        L[name] = (off, w)
        off += w

    for f in range(2 * H // P):
        add(f"wl{f}", C)
    if not bz:
        add("b1td", H)
        add("b1bu", H)
        add("bl", C)
        add("ones", P)
        add("b2bu", 1)
        add("b2td", 1)
    for s in range(NS):
        add(f"pr{s}", G_cap)
    for f in range(F // P):
        add(f"xrootT{f}", G_cap)
    add("croot", G_cap)
    return L, off


def _w8_layout(F, H):
    """Column layout of the fp8 scaled-weight block (rides in piece pa0)."""
    off = 0
    L = {}

    def add(name, w):
        nonlocal off
        L[name] = (off, w)
        off += w

    add("w1td", 2 * H)            # [P, 2, H] f-interleaved pair for DR
    add("w1bu", 2 * H)
    for d in ("bu", "td"):
        for j in range((F + H) // P):
            add(f"w2{d}{j}", H)   # [P, H] chunks as lhsT
    return L, off


def _pack_consts(in_maps, inputs, meta):
    import concourse.mybir as mybir
    import ml_dtypes

    f8 = mybir.dt.np(mybir.dt.float8e4)
    bf16 = ml_dtypes.bfloat16
    H = int(np.asarray(inputs["W_td1"]).shape[1])
    C = int(np.asarray(inputs["W_lin"]).shape[1])
    F, G_cap = meta["F"], meta["G_cap"]
    bz = all(not np.any(np.asarray(inputs[k]))
             for k in ("b_td1", "b_bu1", "b_bu2", "b_td2", "b_lin"))
    L16, W16 = _c16_layout(F, H, C, G_cap, bz)
    L8, W8 = _w8_layout(F, H)
    g = lambda k: np.asarray(inputs[k], dtype=np.float32)

    base16 = np.zeros((P, W16), bf16)

    def put(name, block):
        o, w = L16[name]
        base16[:, o:o + w][tuple(slice(s) for s in block.shape)] = \
            block.astype(bf16)

    for f in range(2 * H // P):
        put(f"wl{f}", g("W_lin")[f * P:(f + 1) * P, :])
    if not bz:
        put("b1td", g("b_td1").reshape(1, H))
        put("b1bu", g("b_bu1").reshape(1, H))
        put("bl", g("b_lin").reshape(1, C))
        put("ones", np.ones((1, P), np.float32))
        put("b2bu", g("b_bu2")[:P, None])
        put("b2td", g("b_td2")[:P, None])

    w8 = np.zeros((P, W8), f8)

    def put8(name, block):
        o, w = L8[name]
        w8[:, o:o + w] = block.astype(f8)

    for d, wn in (("td", "W_td1"), ("bu", "W_bu1")):
        w1 = g(wn) * SC_W                       # [F, H] = [2P, H]
        put8(f"w1{d}", w1.reshape(2, P, H).transpose(1, 0, 2).reshape(P, 2 * H))
    for d, wn in (("bu", "W_bu2"), ("td", "W_td2")):
        w2 = g(wn) * SC_W                       # [F+H, H]
        for j in range((F + H) // P):
            put8(f"w2{d}{j}", w2[j * P:(j + 1) * P, :])

    for m in in_maps:
        c16 = base16.copy()
        pr = m.pop("pr")
        for s in range(NS):
            o, w = L16[f"pr{s}"]
            c16[:, o:o + w] = pr[s * P:(s + 1) * P].astype(bf16)
        xrootT = m.pop("xrootT")
        for f in range(F // P):
            o, w = L16[f"xrootT{f}"]
            c16[:, o:o + w] = xrootT[f * P:(f + 1) * P].astype(bf16)
        o, w = L16["croot"]
        c16[:, o:o + w] = m.pop("croot").astype(bf16)
        m["c16"] = np.ascontiguousarray(c16)
        m["pa0"] = np.ascontiguousarray(np.concatenate([w8, m["pa0"]], axis=1))

    meta["H"], meta["C"] = H, C
    meta["bz"] = bz
    return H


# ----------------------------------------------------------------------------
# Device program
# ----------------------------------------------------------------------------

def _build_program(F, H, C, U, V, W, G_cap, bz=False, repeat=1):
    from contextlib import ExitStack

    import concourse.bacc as bacc
    import concourse.bass as bass  # noqa: F401
    import concourse.mybir as mybir
    import concourse.tile as tile

    dt = mybir.dt
    f32, bf, f8 = dt.float32, dt.bfloat16, dt.float8e4
    nF = F // P
    nW2 = (F + H) // P
    assert F % P == 0 and H == P and nF == 2
    L16, W16 = _c16_layout(F, H, C, G_cap, bz)
    L8, W8 = _w8_layout(F, H)

    T = U + V + W
    U0 = U & ~1                         # bu tile range [U0, T), pair-aligned
    NB_T = T - U0                       # bu tiles
    A_td = U + V
    PA = -(-A_td // 2)
    NXP = T // 2                        # full xt pair blocks
    XPW = nF * 2 * P                    # xt pair block cols
    # piece A: oh_td ((A_td//2) pair blocks + odd single) + xt pairs [0, PA)
    A_OH = (A_td // 2) * 2 * P + (A_td % 2) * P
    ACOLS = A_OH + PA * XPW
    # piece B: oh_bu (pairs + odd single) + xt pairs [PA, NXP) + lone tile
    B_OH = (NB_T // 2) * 2 * P + (NB_T % 2) * P
    B_XS = B_OH + (NXP - PA) * XPW      # offset of the lone last xt tile
    BCOLS = B_XS + (T % 2) * nF * P

    mul, sub, addop, maxop = (
        mybir.AluOpType.mult, mybir.AluOpType.subtract,
        mybir.AluOpType.add, mybir.AluOpType.max)
    Relu, Exp, Ln, Copy = (mybir.ActivationFunctionType.Relu,
                           mybir.ActivationFunctionType.Exp,
                           mybir.ActivationFunctionType.Ln,
                           mybir.ActivationFunctionType.Copy)
    DR = mybir.MatmulPerfMode.DoubleRow

    nc = bacc.Bacc("TRN2", target_bir_lowering=False, debug=False,
                   num_devices=NCORES)

    piece_d = {}
    for s in range(NS):
        piece_d[("a", s)] = nc.dram_tensor(
            f"pa{s}", [P, ACOLS + (W8 if s == 0 else 0)], f8,
            kind="ExternalInput").ap()
        piece_d[("b", s)] = nc.dram_tensor(
            f"pb{s}", [P, BCOLS], f8, kind="ExternalInput").ap()
    c16_d = nc.dram_tensor("c16", [P, W16], bf, kind="ExternalInput").ap()
    out_d = nc.dram_tensor("out", [G_cap, C], f32, kind="ExternalOutput").ap()

    with ExitStack() as ctx:
        tc = ctx.enter_context(tile.TileContext(nc))
        const = ctx.enter_context(tc.tile_pool(
            name="cst", bufs=(1 if repeat == 1 else 3)))
        ppool = ctx.enter_context(tc.tile_pool(
            name="pp", bufs=(2 if repeat == 1 else 3)))
        spool = ctx.enter_context(tc.tile_pool(name="sp", bufs=2))
        psA = ctx.enter_context(tc.tile_pool(name="psA", bufs=3, space="PSUM"))
        psB = ctx.enter_context(tc.tile_pool(name="psB", bufs=3, space="PSUM"))
        psO = ctx.enter_context(tc.tile_pool(name="psO", bufs=1, space="PSUM"))

        # load the one act table containing Exp/Ln/Relu/Copy up-front
        from concourse.hw_specs import get_activation_tables
        need = {Exp, Ln, Relu, Copy}
        for set_id, funcs in enumerate(get_activation_tables(nc.m.arch).values()):
            if need <= funcs:
                nc.scalar.add_instruction(mybir.InstLoadActFuncSet(
                    name=nc.get_next_instruction_name(),
                    act_func_set_id=set_id, ins=[], outs=[]))
                break

        for _rep in range(repeat):
            c16 = const.tile([P, W16], bf, name="c16", tag="c16")

            def C16(name, rows=None):
                o, w = L16[name]
                return c16[:, o:o + w] if rows is None else c16[rows, o:o + w]

            # ---- input DMAs (sync queue): pa0(+w8), pb0, pa1, pb1, c16
            pt = {}
            for key, nm, cols in ((("a", 0), "pa0", ACOLS + W8),
                                  (("b", 0), "pb0", BCOLS),
                                  (("a", 1), "pa1", ACOLS)):
                t = ppool.tile([P, cols], f8, name=nm, tag=nm)
                nc.sync.dma_start(t[:], piece_d[key][:, :])
                pt[key] = t
            nc.sync.dma_start(c16[:], c16_d[:, :])
            t = ppool.tile([P, BCOLS], f8, name="pb1", tag="pb1")
            nc.sync.dma_start(t[:], piece_d[("b", 1)][:, :])
            pt[("b", 1)] = t

            def W8ap(name):
                o, w = L8[name]
                return pt[("a", 0)][:, o:o + w]

            def pieceA(s, off, w):
                base = W8 if s == 0 else 0
                return pt[("a", s)][:, base + off:base + off + w]

            def pieceB(s, off, w):
                return pt[("b", s)][:, off:off + w]

            def xt_pair(s, p, f):
                """xt [P, 2, P] AP for pair p, feature chunk f."""
                if p < PA:
                    ap = pieceA(s, A_OH + p * XPW + f * 2 * P, 2 * P)
                else:
                    ap = pieceB(s, B_OH + (p - PA) * XPW + f * 2 * P, 2 * P)
                return ap.rearrange("p (a b) -> p a b", a=2)

            def xt_single(s, t, f):
                """xt [P, P] AP for tile t, feature chunk f."""
                if t == T - 1 and T % 2:
                    return pieceB(s, B_XS + f * P, P)
                p, a = t // 2, t % 2
                if p < PA:
                    return pieceA(s, A_OH + p * XPW + f * 2 * P + a * P, P)
                return pieceB(s, B_OH + (p - PA) * XPW + f * 2 * P + a * P, P)

            # ---- rT: relu(xrootT) * croot  (Pool engine, SBUF only)
            rT = []
            for f in range(nF):
                tmp = spool.tile([P, G_cap], bf, name=f"rtmp{f}", tag=f"rtmp{f}")
                nc.gpsimd.tensor_scalar(out=tmp[:], in0=C16(f"xrootT{f}"),
                                        scalar1=0.0, scalar2=None, op0=maxop)
                t = spool.tile([P, G_cap], f8, name=f"rT{f}", tag=f"rT{f}")
                nc.gpsimd.tensor_tensor(out=t[:], in0=tmp[:], in1=C16("croot"),
                                        op=mul)
                rT.append(t)

            # ---- per (chunk, dir): stage-1 aggT, copy, l1, relu -> cbt
            cbt = [spool.tile([P, 2 * H], bf, name=f"cbt{s}", tag=f"cbt{s}")
                   for s in range(NS)]
            DI = {"bu": 0, "td": 1}
            CPY = {"td": "act", "bu": "dve"}

            def _copy(eng, dst, src_ap):
                if eng == "act":
                    nc.scalar.activation(dst, src_ap, Copy)
                else:
                    nc.vector.tensor_scalar(out=dst, in0=src_ap, scalar1=0.0,
                                            scalar2=None, op0=addop)

            def _relu_scale(eng, dst, src_ap, scale):
                if eng == "act":
                    nc.scalar.activation(dst, src_ap, Relu, scale=scale)
                else:
                    nc.vector.tensor_scalar(out=dst, in0=src_ap,
                                            scalar1=scale, scalar2=0.0,
                                            op0=mul, op1=maxop)

            aggT_ps = {}
            l1_ps = {}

            def stage1(s, d):
                # aggT psum [p, a, tgt]: f = a*P + p (DR pair layout for l1)
                ps = psA.tile([P, 2, P], f32, name=f"agg{d}{s}", tag="agg")
                aggT_ps[(s, d)] = ps
                if d == "td":
                    pairs = [(j, pieceA(s, j * 2 * P, 2 * P))
                             for j in range(A_td // 2)]
                    odd = A_td % 2
                    odd_t = A_td - 1
                    odd_oh = pieceA(s, (A_td // 2) * 2 * P, P)
                else:
                    pairs = [(U0 // 2 + j, pieceB(s, j * 2 * P, 2 * P))
                             for j in range(NB_T // 2)]
                    odd = NB_T % 2
                    odd_t = T - 1
                    odd_oh = pieceB(s, (NB_T // 2) * 2 * P, P)
                for i, (p, ohap) in enumerate(pairs):
                    oh = ohap.rearrange("p (a b) -> p a b", a=2)
                    for f in range(nF):
                        nc.tensor.matmul(out=ps[:, f, :],
                                         lhsT=xt_pair(s, p, f), rhs=oh,
                                         start=(i == 0),
                                         stop=(i == len(pairs) - 1 and not odd),
                                         perf_mode=DR)
                if odd:
                    for f in range(nF):
                        nc.tensor.matmul(out=ps[:, f, :],
                                         lhsT=xt_single(s, odd_t, f),
                                         rhs=odd_oh,
                                         start=(len(pairs) == 0), stop=True)

            def l1_chain(s, d):
                sb = spool.tile([P, 2, P], f8, name=f"aT{d}{s}",
                                tag=f"aT{d}{s}")
                _copy(CPY[d], sb[:], aggT_ps[(s, d)][:])
                h = psB.tile([P, H], f32, name="hps", tag="psb")
                l1_ps[(s, d)] = h
                nc.tensor.matmul(out=h[:], lhsT=sb[:],
                                 rhs=W8ap(f"w1{d}")
                                 .rearrange("p (a b) -> p a b", a=2),
                                 start=True, stop=bool(bz), perf_mode=DR)
                if not bz:
                    nc.tensor.matmul(out=h[:],
                                     lhsT=C16("ones", rows=slice(0, 1)),
                                     rhs=C16(f"b1{d}", rows=slice(0, 1)),
                                     start=False, stop=True)

            def l1_relu(s, d):
                di = DI[d]
                _relu_scale("dve" if CPY[d] == "act" else "act",
                            cbt[s][:, di * H:(di + 1) * H],
                            l1_ps[(s, d)][:], 1.0 / (SC_OH * SC_W))

            # ---- o2 + tot share one psum tile [P, 4, G_cap]
            # (cols 0-1: o2 m0/m1; cols 2-3: tot bu/td)
            ot_ps = psO.tile([P, 4, G_cap], f32, name="otps", tag="otps")
            o2_ps = ot_ps[:, 0:2, :]
            tot_ps = ot_ps[:, 2:4, :]

            def o2_acc(s, m_):
                nc.tensor.matmul(out=ot_ps[:, m_, :],
                                 lhsT=cbt[s][:, m_ * P:(m_ + 1) * P],
                                 rhs=C16(f"pr{s}"), start=(s == 0),
                                 stop=(s == NS - 1))

            for d in ("td", "bu"):
                stage1(0, d)
            for d in ("td", "bu"):
                l1_chain(0, d)
                l1_relu(0, d)
            for d in ("bu", "td"):
                stage1(1, d)
            for d in ("bu", "td"):
                l1_chain(1, d)
                l1_relu(1, d)
            for m_ in range(2):
                for s in range(NS):
                    o2_acc(s, m_)

            # single psum->sbuf copy for both o2 halves (DVE)
            o2_sb = spool.tile([P, 2, G_cap], f8, name="o2sb", tag="o2sb")
            _copy("dve", o2_sb[:], o2_ps)

            # ---- tot[d] = relu((W2s_d^T [rT; o2_d]) / SC_W + b2_d)
            for di, d in enumerate(("bu", "td")):
                for j in range(nW2):
                    rhs_t = rT[j][:] if j < nF else o2_sb[:, di, :]
                    nc.tensor.matmul(out=ot_ps[:, 2 + di, :],
                                     lhsT=W8ap(f"w2{d}{j}"), rhs=rhs_t,
                                     start=(j == 0), stop=(j == nW2 - 1))
            tot_sb = spool.tile([P, 2, G_cap], bf, name="totsb", tag="totsb")
            if bz:
                nc.scalar.activation(tot_sb[:], tot_ps, Relu,
                                     scale=1.0 / SC_W)
            else:
                for di, d in enumerate(("bu", "td")):
                    nc.scalar.activation(tot_sb[:, di, :], ot_ps[:, 2 + di, :],
                                         Relu, scale=1.0 / SC_W,
                                         bias=C16(f"b2{d}"))

            # ---- logits + log_softmax
            lg = psO.tile([G_cap, C], f32, name="lgps", tag="lgps")
            for di in range(2):
                nc.tensor.matmul(out=lg[:], lhsT=tot_sb[:, di, :G_cap],
                                 rhs=C16(f"wl{di}"), start=(di == 0),
                                 stop=(bz and di == 1))
            if not bz:
                nc.tensor.matmul(out=lg[:],
                                 lhsT=C16("ones", rows=slice(0, 1))[:, :G_cap],
                                 rhs=C16("bl", rows=slice(0, 1)),
                                 start=False, stop=True)
            ez = spool.tile([G_cap, C], f32, name="ez", tag="ez")
            se = spool.tile([G_cap, 1], f32, name="se", tag="se")
            nc.scalar.activation(ez[:], lg[:], Exp, accum_out=se[:])
            lse = spool.tile([G_cap, 1], f32, name="lse", tag="lse")
            nc.scalar.activation(lse[:], se[:], Ln)
            res = spool.tile([G_cap, C], f32, name="res", tag="res")
            nc.vector.tensor_scalar(out=res[:], in0=lg[:], scalar1=lse[:],
                                    scalar2=None, op0=sub)
            # out DMA from the Act queue (res lands right after Act's ln, so
            # the wait barely blocks it) -- keeps the SP queue a pure input
            # stream so the next repetition's input DMAs issue while this rep
            # computes
            if repeat == 1:
                nc.sync.dma_start(out_d[:], res[:])
            else:
                nc.scalar.dma_start(out_d[:], res[:])

    nc.compile()
    return nc


_PROG_CACHE = {}


def _prepare_maps(inputs):
    in_maps, meta = _preprocess(inputs["x"], inputs["edge_index"],
                                inputs["batch"], inputs["num_graphs"])
    _pack_consts(in_maps, inputs, meta)
    return in_maps, meta


def _prog_key(meta):
    return (meta["F"], meta["H"], meta["C"], meta["U"], meta["V"], meta["W"],
            meta["G_cap"], meta["bz"])


def _prepare(inputs):
    in_maps, meta = _prepare_maps(inputs)
    key = _prog_key(meta)
    if key not in _PROG_CACHE:
        _PROG_CACHE[key] = _build_program(*key)
    return _PROG_CACHE[key], in_maps, meta


def kernel(**inputs):
    from concourse.bass_utils import run_bass_kernel_spmd

    nc, in_maps, meta = _prepare(inputs)
    res = run_bass_kernel_spmd(nc, in_maps, list(range(NCORES)))
    G = meta["G"]
    cog, gl = meta["core_of_graph"], meta["glocal"]
    out = np.empty((G, meta["C"]), np.float32)
    for g in range(G):
        out[g] = res.results[cog[g]]["out"][gl[g]]
    return out


# revision 27
# speedup vs baseline: 14.0362x; 1.6912x over previous
"""BiGCN (nn_BiGCN_52716428591487) Trainium2 kernel, v3.

Math: the model's output is log_softmax(cat(l2_bu[root], l2_td[root]) @ W_lin + b).
Only the layer-2 GCN values AT THE ROOT NODES matter, and GCNConv is linear in
its input features, so the whole network collapses to:

  agg1_d[v]  = sum_{e -> v} coef_d(e) * x[nbr(e)]            (v in S; self-loops
               folded into the edge list with coef dinv_d[v]^2)
  l1_d[v]    = agg1_d[v] @ W_d1 + b_d1
  out2_l1[g] = sum_{s in S_g} Pr[s, g] * relu(l1_d[s])       (layer-2 agg)
  out2_R[g]  = c_g * relu(x[root_g])                          (root-feature block
               collapses: Pr is block-diagonal by graph, c_g = sum_s Pr[s, g])
  pb/pt[g]   = relu(W2_d^T [out2_R; out2_l1_d] + b_2)
  out[g]     = log_softmax([pb, pt][g] @ W_lin + b_lin)

where S = {sources of root-incident edges} + {roots} (~1.7k of 50k nodes).

Host does index-only preprocessing (degrees, edge selection, dedup/gather
tables, the one-hot scatter matrices, Pr) plus dtype casts/layout; the device
does every arithmetic op that touches x or the weights.

Device shape: per (chunk, dir), stage-1 computes aggT = xt^T @ onehot
directly in PSUM (lhsT = gathered x rows, rhs = one-hot with coefs), so no
transposes are needed; gathered rows are deduplicated across BOTH directions
per chunk ([td-only | shared | bu-only] tile layout, one fp8 copy of x rows
serving both one-hots); l1 is a single fp8 DoubleRow matmul; the head is a
short chain of tiny matmuls + activations on [G_cap]-wide data.
W1/W2 ship as fp8 (x16 host cast-scale, descaled inside the relu
activations); one-hot coefs are x8.  5 input DMAs per core.

Sharding: graph-data parallel over 8 cores; the host concatenates the
per-core [G_cap, C] outputs.
"""

import numpy as np

P = 128
NCORES = 8
NS = 2           # slot chunks of 128 target slots per core
SC_OH = 8.0      # host scale on one-hot coefs
SC_W = 16.0      # host scale on fp8 W1/W2


def _roundup(a, m):
    return -(-int(a) // m) * m


# ----------------------------------------------------------------------------
# Host preprocessing: index-only work + gather/pack tables
# ----------------------------------------------------------------------------

def _preprocess(x, edge_index, batch, num_graphs):
    import concourse.mybir as mybir

    f8 = mybir.dt.np(mybir.dt.float8e4)

    x = np.ascontiguousarray(np.asarray(x), dtype=np.float32)
    ei = np.asarray(edge_index)
    batch = np.asarray(batch).astype(np.int64)
    G = int(np.asarray(num_graphs))
    N, F = x.shape
    nF = F // P
    src = ei[0].astype(np.int64)
    dst = ei[1].astype(np.int64)

    assert np.all(np.diff(batch) >= 0), "batch must be sorted (contiguous graphs)"
    roots = np.searchsorted(batch, np.arange(G, dtype=np.int64))  # segment_min

    deg_td = 1.0 + np.bincount(dst, minlength=N).astype(np.float64)
    deg_bu = 1.0 + np.bincount(src, minlength=N).astype(np.float64)
    dinv_td = (1.0 / np.sqrt(deg_td)).astype(np.float32)
    dinv_bu = (1.0 / np.sqrt(deg_bu)).astype(np.float32)

    G_cap = max(-(-G // NCORES), 1)

    # S: sources of root-incident edges + roots
    is_root = np.zeros(N, bool)
    is_root[roots] = True
    rmask = is_root[dst]
    r_src, r_dst = src[rmask], dst[rmask]
    r_coef = dinv_td[r_src] * dinv_td[r_dst]

    s_nodes = np.unique(np.concatenate([r_src, roots]))  # sorted
    s_graph = batch[s_nodes]
    s_count_g = np.bincount(s_graph, minlength=G)
    assert s_count_g.max() <= P, "graph S-set exceeds one chunk"

    in_s = np.zeros(N, bool)
    in_s[s_nodes] = True

    def _dir_edges(tgt_nodes, row_nodes, dinv):
        m = in_s[tgt_nodes]
        tg, rw = tgt_nodes[m], row_nodes[m]
        cf = (dinv[rw] * dinv[tg]).astype(np.float32)
        tg = np.concatenate([tg, s_nodes])          # self loops
        rw = np.concatenate([rw, s_nodes])
        cf = np.concatenate([cf, (dinv[s_nodes] ** 2).astype(np.float32)])
        return tg, rw, cf, batch[tg]

    E_td = _dir_edges(dst, src, dinv_td)
    E_bu = _dir_edges(src, dst, dinv_bu)

    # per-graph unique-row category counts (td-only / shared / bu-only)
    cat_g = np.zeros((G, 3), np.int64)
    uniq_rows_td = [None] * G
    uniq_rows_bu = [None] * G

    def _per_graph_rows(E):
        tg, rw, cf, eg = E
        order = np.argsort(eg, kind="stable")
        egs, rws = eg[order], rw[order]
        bnd = np.searchsorted(egs, np.arange(G + 1))
        return [np.unique(rws[bnd[g]:bnd[g + 1]]) for g in range(G)]

    uniq_rows_td = _per_graph_rows(E_td)
    uniq_rows_bu = _per_graph_rows(E_bu)
    for g in range(G):
        sh = np.intersect1d(uniq_rows_td[g], uniq_rows_bu[g],
                            assume_unique=True)
        cat_g[g, 1] = len(sh)
        cat_g[g, 0] = len(uniq_rows_td[g]) - len(sh)
        cat_g[g, 2] = len(uniq_rows_bu[g]) - len(sh)

    gw = cat_g.sum(axis=1)

    # graph -> (core, chunk) bin: direct greedy over NCORES*NS bins on
    # per-category loads, then local-search refinement minimizing the global
    # per-category maxima (which set the padded tile counts U/V/W).
    NB = NCORES * NS
    bin_of_graph = np.empty(G, np.int64)
    bcount = np.zeros(NB, np.int64)       # graphs per bin (core cap G_cap)
    bfill = np.zeros(NB, np.int64)        # slots per bin (cap P)
    bld = np.zeros((NB, 3))
    ccount = np.zeros(NCORES, np.int64)

    def _core_ok(b, extra=1):
        return ccount[b // NS] + extra <= G_cap * 1  # per-core graph cap

    for g in np.argsort(-gw, kind="stable"):
        cands = [b for b in range(NB)
                 if bfill[b] + s_count_g[g] <= P and ccount[b // NS] < G_cap]
        b = min(cands, key=lambda bb: ((bld[bb] + cat_g[g]).max(),
                                       bld[bb].sum()))
        bin_of_graph[g] = b
        bcount[b] += 1
        bfill[b] += s_count_g[g]
        bld[b] += cat_g[g]
        ccount[b // NS] += 1

    def _cost(loads):
        mx = loads.max(axis=0)
        u, v, w = (int(-(-m // P)) for m in mx)
        u0 = u & ~1
        t, a = u + v + w, u + v
        return (2 * t + a + (t - u0), mx.sum())

    # local search: single moves + pairwise swaps
    for _sweep in range(6):
        improved = False
        order = np.argsort(-(bld.max(axis=1)))
        for g in range(G):
            b0 = bin_of_graph[g]
            for b1 in range(NB):
                if b1 == b0:
                    continue
                if bfill[b1] + s_count_g[g] <= P and \
                        (b1 // NS == b0 // NS or ccount[b1 // NS] < G_cap):
                    new = bld.copy()
                    new[b0] -= cat_g[g]
                    new[b1] += cat_g[g]
                    if _cost(new) < _cost(bld):
                        bld = new
                        bfill[b0] -= s_count_g[g]
                        bfill[b1] += s_count_g[g]
                        ccount[b0 // NS] -= 1
                        ccount[b1 // NS] += 1
                        bcount[b0] -= 1
                        bcount[b1] += 1
                        bin_of_graph[g] = b1
                        b0 = b1
                        improved = True
        # pairwise swaps
        for g1 in range(G):
            for g2 in range(g1 + 1, G):
                b1, b2 = bin_of_graph[g1], bin_of_graph[g2]
                if b1 == b2:
                    continue
                if bfill[b1] - s_count_g[g1] + s_count_g[g2] > P or \
                        bfill[b2] - s_count_g[g2] + s_count_g[g1] > P:
                    continue
                new = bld.copy()
                new[b1] += cat_g[g2] - cat_g[g1]
                new[b2] += cat_g[g1] - cat_g[g2]
                if _cost(new) < _cost(bld):
                    bld = new
                    bfill[b1] += s_count_g[g2] - s_count_g[g1]
                    bfill[b2] += s_count_g[g1] - s_count_g[g2]
                    bin_of_graph[g1], bin_of_graph[g2] = b2, b1
                    improved = True
        if not improved:
            break

    core_of_graph = bin_of_graph // NS
    chunk_of_graph = bin_of_graph % NS
    glocal = np.empty(G, np.int64)
    counts = np.zeros(NCORES, np.int64)
    for g in range(G):
        glocal[g] = counts[core_of_graph[g]]
        counts[core_of_graph[g]] += 1

    # slot layout per bin
    s_lookup = np.full(N, -1, np.int64)   # node -> core slot (chunk*P + i)
    binfill = np.zeros(NB, np.int64)
    for g in range(G):
        b = bin_of_graph[g]
        s = chunk_of_graph[g]
        idx = s_nodes[s_graph == g]
        s_lookup[idx] = s * P + binfill[b] + np.arange(len(idx))
        binfill[b] += s_count_g[g]
    assert (binfill <= P).all()

    # ---- per (core, chunk): union rows in [td_only | shared | bu_only]
    # category layout; per dir entry lists reference row positions
    rows_cat = {}   # (c, s) -> (rows_to, rows_sh, rows_bo)
    for c in range(NCORES):
        for s in range(NS):
            gs = np.flatnonzero((core_of_graph == c) & (chunk_of_graph == s))
            rt = (np.concatenate([uniq_rows_td[g] for g in gs])
                  if len(gs) else np.empty(0, np.int64))
            rb = (np.concatenate([uniq_rows_bu[g] for g in gs])
                  if len(gs) else np.empty(0, np.int64))
            sh = np.intersect1d(rt, rb, assume_unique=True)
            to = np.setdiff1d(rt, sh, assume_unique=True)
            bo = np.setdiff1d(rb, sh, assume_unique=True)
            rows_cat[(c, s)] = (to, sh, bo)

    # uniform category tile counts (no parity padding; bu's pair range
    # starts at the even floor U0 and both dirs may end with an odd single)
    U = _roundup(max(len(rows_cat[k][0]) for k in rows_cat), P) // P
    V = _roundup(max(len(rows_cat[k][1]) for k in rows_cat), P) // P
    W = _roundup(max(len(rows_cat[k][2]) for k in rows_cat), P) // P
    T = U + V + W                                  # xt tiles per chunk
    A_td = U + V                                   # td tile range [0, A_td)
    PA = -(-A_td // 2)                             # xt pairs in piece A

    # entry maps per (c, s, d): (row_pos_in_chunk_layout, tgt_local, coef)
    ents = {}
    for d, E in (("td", E_td), ("bu", E_bu)):
        tg, rw, cf, eg = E
        slot = s_lookup[tg]
        assert (slot >= 0).all()
        core = core_of_graph[eg]
        chunk = slot // P
        for c in range(NCORES):
            for s in range(NS):
                m = (core == c) & (chunk == s)
                rw_m, cf_m = rw[m], cf[m]
                tloc = (slot[m] - s * P).astype(np.int64)
                to, sh, bo = rows_cat[(c, s)]
                # map node id -> row position in the chunk layout
                pos = np.full(len(rw_m), -1, np.int64)
                for base, cat in ((0, to), (U * P, sh), ((U + V) * P, bo)):
                    idx = np.searchsorted(cat, rw_m)
                    idx_c = np.clip(idx, 0, max(len(cat) - 1, 0))
                    hit = (len(cat) > 0) & (cat[idx_c] == rw_m) if len(cat) \
                        else np.zeros(len(rw_m), bool)
                    pos = np.where(hit, base + idx_c, pos)
                assert (pos >= 0).all()
                ents[(c, s, d)] = (pos, tloc, cf_m)

    # layer-2 aggregation matrix Pr[core, slot, glocal] and root tables
    r_graph = batch[r_dst]
    S_cap = NS * P
    Pr = np.zeros((NCORES, S_cap, G_cap), np.float32)
    np.add.at(Pr, (core_of_graph[r_graph], s_lookup[r_src], glocal[r_graph]),
              r_coef)
    np.add.at(Pr, (core_of_graph[np.arange(G)], s_lookup[roots], glocal),
              dinv_td[roots] ** 2)

    # ---- pack per-core pieces
    # piece A (per chunk): oh_td blocks + xt pairs [0, PA)
    # piece B: oh_bu blocks + xt pairs [PA, T/2)
    U0 = U & ~1
    NXP = T // 2

    def _pack(c, s):
        to, sh, bo = rows_cat[(c, s)]
        # x rows in chunk layout [T*P, F] fp8
        xg = np.zeros((T * P, F), f8)
        xg[:len(to)] = x[to].astype(f8)
        xg[U * P:U * P + len(sh)] = x[sh].astype(f8)
        xg[(U + V) * P:(U + V) * P + len(bo)] = x[bo].astype(f8)
        xg = xg.reshape(T, P, F)
        # xt pair blocks [P, nF, 2, P] -> [P, nF*2*P] per pair
        xp = xg[:2 * NXP].reshape(NXP, 2, P, nF, P).transpose(2, 0, 3, 1, 4) \
            .reshape(P, NXP, nF * 2 * P)

        def _oh(d, tile_lo, n_tiles):
            posm, tloc, cf = ents[(c, s, d)]
            oh = np.zeros((n_tiles, P, P), np.float32)
            t_idx = posm // P - tile_lo
            assert (t_idx >= 0).all() and (t_idx < n_tiles).all()
            np.add.at(oh, (t_idx, posm % P, tloc), cf * SC_OH)
            return oh.astype(f8)

        def _blocks(oh, n_tiles):
            out = [np.stack([oh[2 * j], oh[2 * j + 1]], axis=1)
                   .reshape(P, 2 * P) for j in range(n_tiles // 2)]
            if n_tiles % 2:
                out.append(oh[n_tiles - 1])
            return out

        td_blocks = _blocks(_oh("td", 0, A_td), A_td)
        bu_blocks = _blocks(_oh("bu", U0, T - U0), T - U0)
        a_parts = td_blocks + [xp[:, :PA].reshape(P, -1)]
        b_parts = bu_blocks + [xp[:, PA:].reshape(P, -1)]
        if T % 2:
            # last lone tile stored as a single block [P, nF*P] in piece B
            xs = np.ascontiguousarray(xg[T - 1].reshape(P, nF, P)
                                      .reshape(P, nF * P))
            b_parts.append(xs)
        pieceA = np.concatenate(a_parts, axis=1)
        pieceB = np.concatenate(b_parts, axis=1)
        return (np.ascontiguousarray(pieceA), np.ascontiguousarray(pieceB))

    in_maps = []
    for c in range(NCORES):
        m = {}
        for s in range(NS):
            a, b = _pack(c, s)
            m[f"pa{s}"] = a
            m[f"pb{s}"] = b
        m["pr"] = Pr[c]
        gs = np.flatnonzero(core_of_graph == c)
        xrootT = np.zeros((F, G_cap), np.float32)
        xrootT[:, glocal[gs]] = x[roots[gs]].T
        m["xrootT"] = xrootT
        m["croot"] = np.tile(Pr[c].sum(axis=0, dtype=np.float64)
                             .astype(np.float32), (P, 1))
        in_maps.append(m)

    meta = dict(F=F, U=U, V=V, W=W, G_cap=G_cap, counts=counts, G=G,
                core_of_graph=core_of_graph, glocal=glocal)
    return in_maps, meta


def _c16_layout(F, H, C, G_cap, bz):
    """Column layout of the bf16 constant matrix [P, W16]."""
    off = 0
    L = {}

    def add(name, w):
        nonlocal off
        L[name] = (off, w)
        off += w

    for f in range(2 * H // P):
        add(f"wl{f}", C)
    if not bz:
        add("b1td", H)
        add("b1bu", H)
        add("bl", C)
        add("ones", P)
        add("b2bu", 1)
        add("b2td", 1)
    for s in range(NS):
        add(f"pr{s}", G_cap)
    for f in range(F // P):
        add(f"xrootT{f}", G_cap)
    add("croot", G_cap)
    return L, off


def _w8_layout(F, H):
    """Column layout of the fp8 scaled-weight block (rides in piece pa0)."""
    off = 0
    L = {}

    def add(name, w):
        nonlocal off
        L[name] = (off, w)
        off += w

    add("w1td", 2 * H)            # [P, 2, H] f-interleaved pair for DR
    add("w1bu", 2 * H)
    for d in ("bu", "td"):
        for j in range((F + H) // P):
            add(f"w2{d}{j}", H)   # [P, H] chunks as lhsT
    return L, off


def _pack_consts(in_maps, inputs, meta):
    import concourse.mybir as mybir
    import ml_dtypes

    f8 = mybir.dt.np(mybir.dt.float8e4)
    bf16 = ml_dtypes.bfloat16
    H = int(np.asarray(inputs["W_td1"]).shape[1])
    C = int(np.asarray(inputs["W_lin"]).shape[1])
    F, G_cap = meta["F"], meta["G_cap"]
    bz = all(not np.any(np.asarray(inputs[k]))
             for k in ("b_td1", "b_bu1", "b_bu2", "b_td2", "b_lin"))
    L16, W16 = _c16_layout(F, H, C, G_cap, bz)
    L8, W8 = _w8_layout(F, H)
    g = lambda k: np.asarray(inputs[k], dtype=np.float32)

    base16 = np.zeros((P, W16), bf16)

    def put(name, block):
        o, w = L16[name]
        base16[:, o:o + w][tuple(slice(s) for s in block.shape)] = \
            block.astype(bf16)

    for f in range(2 * H // P):
        put(f"wl{f}", g("W_lin")[f * P:(f + 1) * P, :])
    if not bz:
        put("b1td", g("b_td1").reshape(1, H))
        put("b1bu", g("b_bu1").reshape(1, H))
        put("bl", g("b_lin").reshape(1, C))
        put("ones", np.ones((1, P), np.float32))
        put("b2bu", g("b_bu2")[:P, None])
        put("b2td", g("b_td2")[:P, None])

    w8 = np.zeros((P, W8), f8)

    def put8(name, block):
        o, w = L8[name]
        w8[:, o:o + w] = block.astype(f8)

    for d, wn in (("td", "W_td1"), ("bu", "W_bu1")):
        w1 = g(wn) * SC_W                       # [F, H] = [2P, H]
        put8(f"w1{d}", w1.reshape(2, P, H).transpose(1, 0, 2).reshape(P, 2 * H))
    for d, wn in (("bu", "W_bu2"), ("td", "W_td2")):
        w2 = g(wn) * SC_W                       # [F+H, H]
        for j in range((F + H) // P):
            put8(f"w2{d}{j}", w2[j * P:(j + 1) * P, :])

    for m in in_maps:
        c16 = base16.copy()
        pr = m.pop("pr")
        for s in range(NS):
            o, w = L16[f"pr{s}"]
            c16[:, o:o + w] = pr[s * P:(s + 1) * P].astype(bf16)
        xrootT = m.pop("xrootT")
        for f in range(F // P):
            o, w = L16[f"xrootT{f}"]
            c16[:, o:o + w] = xrootT[f * P:(f + 1) * P].astype(bf16)
        o, w = L16["croot"]
        c16[:, o:o + w] = m.pop("croot").astype(bf16)
        m["c16"] = np.ascontiguousarray(c16)
        m["pa0"] = np.ascontiguousarray(np.concatenate([w8, m["pa0"]], axis=1))

    meta["H"], meta["C"] = H, C
    meta["bz"] = bz
    return H


# ----------------------------------------------------------------------------
# Device program
# ----------------------------------------------------------------------------

def _build_program(F, H, C, U, V, W, G_cap, bz=False, repeat=1):
    from contextlib import ExitStack

    import concourse.bacc as bacc
    import concourse.bass as bass  # noqa: F401
    import concourse.mybir as mybir
    import concourse.tile as tile

    dt = mybir.dt
    f32, bf, f8 = dt.float32, dt.bfloat16, dt.float8e4
    nF = F // P
    nW2 = (F + H) // P
    assert F % P == 0 and H == P and nF == 2
    L16, W16 = _c16_layout(F, H, C, G_cap, bz)
    L8, W8 = _w8_layout(F, H)

    T = U + V + W
    U0 = U & ~1                         # bu tile range [U0, T), pair-aligned
    NB_T = T - U0                       # bu tiles
    A_td = U + V
    PA = -(-A_td // 2)
    NXP = T // 2                        # full xt pair blocks
    XPW = nF * 2 * P                    # xt pair block cols
    # piece A: oh_td ((A_td//2) pair blocks + odd single) + xt pairs [0, PA)
    A_OH = (A_td // 2) * 2 * P + (A_td % 2) * P
    ACOLS = A_OH + PA * XPW
    # piece B: oh_bu (pairs + odd single) + xt pairs [PA, NXP) + lone tile
    B_OH = (NB_T // 2) * 2 * P + (NB_T % 2) * P
    B_XS = B_OH + (NXP - PA) * XPW      # offset of the lone last xt tile
    BCOLS = B_XS + (T % 2) * nF * P

    mul, sub, addop, maxop = (
        mybir.AluOpType.mult, mybir.AluOpType.subtract,
        mybir.AluOpType.add, mybir.AluOpType.max)
    Relu, Exp, Ln, Copy = (mybir.ActivationFunctionType.Relu,
                           mybir.ActivationFunctionType.Exp,
                           mybir.ActivationFunctionType.Ln,
                           mybir.ActivationFunctionType.Copy)
    DR = mybir.MatmulPerfMode.DoubleRow

    nc = bacc.Bacc("TRN2", target_bir_lowering=False, debug=False,
                   num_devices=NCORES)

    piece_d = {}
    for s in range(NS):
        piece_d[("a", s)] = nc.dram_tensor(
            f"pa{s}", [P, ACOLS + (W8 if s == 0 else 0)], f8,
            kind="ExternalInput").ap()
        piece_d[("b", s)] = nc.dram_tensor(
            f"pb{s}", [P, BCOLS], f8, kind="ExternalInput").ap()
    c16_d = nc.dram_tensor("c16", [P, W16], bf, kind="ExternalInput").ap()
    out_d = nc.dram_tensor("out", [G_cap, C], f32, kind="ExternalOutput").ap()

    with ExitStack() as ctx:
        tc = ctx.enter_context(tile.TileContext(nc))
        const = ctx.enter_context(tc.tile_pool(
            name="cst", bufs=(1 if repeat == 1 else 3)))
        ppool = ctx.enter_context(tc.tile_pool(
            name="pp", bufs=(2 if repeat == 1 else 3)))
        spool = ctx.enter_context(tc.tile_pool(name="sp", bufs=2))
        psA = ctx.enter_context(tc.tile_pool(name="psA", bufs=3, space="PSUM"))
        psB = ctx.enter_context(tc.tile_pool(name="psB", bufs=3, space="PSUM"))
        psO = ctx.enter_context(tc.tile_pool(name="psO", bufs=1, space="PSUM"))

        # load the one act table containing Exp/Ln/Relu/Copy up-front
        from concourse.hw_specs import get_activation_tables
        need = {Exp, Ln, Relu, Copy}
        for set_id, funcs in enumerate(get_activation_tables(nc.m.arch).values()):
            if need <= funcs:
                nc.scalar.add_instruction(mybir.InstLoadActFuncSet(
                    name=nc.get_next_instruction_name(),
                    act_func_set_id=set_id, ins=[], outs=[]))
                break

        for _rep in range(repeat):
            c16 = const.tile([P, W16], bf, name="c16", tag="c16")

            def C16(name, rows=None):
                o, w = L16[name]
                return c16[:, o:o + w] if rows is None else c16[rows, o:o + w]

            # ---- input DMAs (sync queue): pa0(+w8), pb0, pa1, pb1, c16
            pt = {}
            for key, nm, cols in ((("a", 0), "pa0", ACOLS + W8),
                                  (("b", 0), "pb0", BCOLS),
                                  (("a", 1), "pa1", ACOLS)):
                t = ppool.tile([P, cols], f8, name=nm, tag=nm)
                nc.sync.dma_start(t[:], piece_d[key][:, :])
                pt[key] = t
            nc.sync.dma_start(c16[:], c16_d[:, :])
            t = ppool.tile([P, BCOLS], f8, name="pb1", tag="pb1")
            nc.sync.dma_start(t[:], piece_d[("b", 1)][:, :])
            pt[("b", 1)] = t

            def W8ap(name):
                o, w = L8[name]
                return pt[("a", 0)][:, o:o + w]

            def pieceA(s, off, w):
                base = W8 if s == 0 else 0
                return pt[("a", s)][:, base + off:base + off + w]

            def pieceB(s, off, w):
                return pt[("b", s)][:, off:off + w]

            def xt_pair(s, p, f):
                """xt [P, 2, P] AP for pair p, feature chunk f."""
                if p < PA:
                    ap = pieceA(s, A_OH + p * XPW + f * 2 * P, 2 * P)
                else:
                    ap = pieceB(s, B_OH + (p - PA) * XPW + f * 2 * P, 2 * P)
                return ap.rearrange("p (a b) -> p a b", a=2)

            def xt_single(s, t, f):
                """xt [P, P] AP for tile t, feature chunk f."""
                if t == T - 1 and T % 2:
                    return pieceB(s, B_XS + f * P, P)
                p, a = t // 2, t % 2
                if p < PA:
                    return pieceA(s, A_OH + p * XPW + f * 2 * P + a * P, P)
                return pieceB(s, B_OH + (p - PA) * XPW + f * 2 * P + a * P, P)

            # ---- rT: relu(xrootT) * croot  (Pool engine, SBUF only)
            rT = []
            for f in range(nF):
                tmp = spool.tile([P, G_cap], bf, name=f"rtmp{f}", tag=f"rtmp{f}")
                nc.gpsimd.tensor_scalar(out=tmp[:], in0=C16(f"xrootT{f}"),
                                        scalar1=0.0, scalar2=None, op0=maxop)
                t = spool.tile([P, G_cap], f8, name=f"rT{f}", tag=f"rT{f}")
                nc.gpsimd.tensor_tensor(out=t[:], in0=tmp[:], in1=C16("croot"),
                                        op=mul)
                rT.append(t)

            # ---- per (chunk, dir): stage-1 aggT, copy, l1, relu -> cbt
            cbt = [spool.tile([P, 2 * H], bf, name=f"cbt{s}", tag=f"cbt{s}")
                   for s in range(NS)]
            DI = {"bu": 0, "td": 1}
            CPY = {"td": "act", "bu": "dve"}

            def _copy(eng, dst, src_ap):
                if eng == "act":
                    nc.scalar.activation(dst, src_ap, Copy)
                else:
                    nc.vector.tensor_scalar(out=dst, in0=src_ap, scalar1=0.0,
                                            scalar2=None, op0=addop)

            def _relu_scale(eng, dst, src_ap, scale):
                if eng == "act":
                    nc.scalar.activation(dst, src_ap, Relu, scale=scale)
                else:
                    nc.vector.tensor_scalar(out=dst, in0=src_ap,
                                            scalar1=scale, scalar2=0.0,
                                            op0=mul, op1=maxop)

            aggT_ps = {}
            l1_ps = {}

            def stage1(s, d):
                # aggT psum [p, a, tgt]: f = a*P + p (DR pair layout for l1)
                ps = psA.tile([P, 2, P], f32, name=f"agg{d}{s}", tag="agg")
                aggT_ps[(s, d)] = ps
                if d == "td":
                    pairs = [(j, pieceA(s, j * 2 * P, 2 * P))
                             for j in range(A_td // 2)]
                    odd = A_td % 2
                    odd_t = A_td - 1
                    odd_oh = pieceA(s, (A_td // 2) * 2 * P, P)
                else:
                    pairs = [(U0 // 2 + j, pieceB(s, j * 2 * P, 2 * P))
                             for j in range(NB_T // 2)]
                    odd = NB_T % 2
                    odd_t = T - 1
                    odd_oh = pieceB(s, (NB_T // 2) * 2 * P, P)
                for i, (p, ohap) in enumerate(pairs):
                    oh = ohap.rearrange("p (a b) -> p a b", a=2)
                    for f in range(nF):
                        nc.tensor.matmul(out=ps[:, f, :],
                                         lhsT=xt_pair(s, p, f), rhs=oh,
                                         start=(i == 0),
                                         stop=(i == len(pairs) - 1 and not odd),
                                         perf_mode=DR)
                if odd:
                    for f in range(nF):
                        nc.tensor.matmul(out=ps[:, f, :],
                                         lhsT=xt_single(s, odd_t, f),
                                         rhs=odd_oh,
                                         start=(len(pairs) == 0), stop=True)

            def l1_chain(s, d):
                sb = spool.tile([P, 2, P], f8, name=f"aT{d}{s}",
                                tag=f"aT{d}{s}")
                _copy(CPY[d], sb[:], aggT_ps[(s, d)][:])
                h = psB.tile([P, H], f32, name="hps", tag="psb")
                l1_ps[(s, d)] = h
                nc.tensor.matmul(out=h[:], lhsT=sb[:],
                                 rhs=W8ap(f"w1{d}")
                                 .rearrange("p (a b) -> p a b", a=2),
                                 start=True, stop=bool(bz), perf_mode=DR)
                if not bz:
                    nc.tensor.matmul(out=h[:],
                                     lhsT=C16("ones", rows=slice(0, 1)),
                                     rhs=C16(f"b1{d}", rows=slice(0, 1)),
                                     start=False, stop=True)

            def l1_relu(s, d):
                di = DI[d]
                _relu_scale("dve" if CPY[d] == "act" else "act",
                            cbt[s][:, di * H:(di + 1) * H],
                            l1_ps[(s, d)][:], 1.0 / (SC_OH * SC_W))

            # ---- o2 + tot share one psum tile [P, 4, G_cap]
            # (cols 0-1: o2 m0/m1; cols 2-3: tot bu/td)
            ot_ps = psO.tile([P, 4, G_cap], f32, name="otps", tag="otps")
            o2_ps = ot_ps[:, 0:2, :]
            tot_ps = ot_ps[:, 2:4, :]

            def o2_acc(s, m_):
                nc.tensor.matmul(out=ot_ps[:, m_, :],
                                 lhsT=cbt[s][:, m_ * P:(m_ + 1) * P],
                                 rhs=C16(f"pr{s}"), start=(s == 0),
                                 stop=(s == NS - 1))

            for d in ("td", "bu"):
                stage1(0, d)
            for d in ("td", "bu"):
                l1_chain(0, d)
                l1_relu(0, d)
            for d in ("bu", "td"):
                stage1(1, d)
            for d in ("bu", "td"):
                l1_chain(1, d)
                l1_relu(1, d)
            for m_ in range(2):
                for s in range(NS):
                    o2_acc(s, m_)

            # single psum->sbuf copy for both o2 halves (DVE)
            o2_sb = spool.tile([P, 2, G_cap], f8, name="o2sb", tag="o2sb")
            _copy("dve", o2_sb[:], o2_ps)

            # ---- tot[d] = relu((W2s_d^T [rT; o2_d]) / SC_W + b2_d)
            for di, d in enumerate(("bu", "td")):
                for j in range(nW2):
                    rhs_t = rT[j][:] if j < nF else o2_sb[:, di, :]
                    nc.tensor.matmul(out=ot_ps[:, 2 + di, :],
                                     lhsT=W8ap(f"w2{d}{j}"), rhs=rhs_t,
                                     start=(j == 0), stop=(j == nW2 - 1))
            tot_sb = spool.tile([P, 2, G_cap], bf, name="totsb", tag="totsb")
            if bz:
                nc.scalar.activation(tot_sb[:], tot_ps, Relu,
                                     scale=1.0 / SC_W)
            else:
                for di, d in enumerate(("bu", "td")):
                    nc.scalar.activation(tot_sb[:, di, :], ot_ps[:, 2 + di, :],
                                         Relu, scale=1.0 / SC_W,
                                         bias=C16(f"b2{d}"))

            # ---- logits + log_softmax
            lg = psO.tile([G_cap, C], f32, name="lgps", tag="lgps")
            for di in range(2):
                nc.tensor.matmul(out=lg[:], lhsT=tot_sb[:, di, :G_cap],
                                 rhs=C16(f"wl{di}"), start=(di == 0),
                                 stop=(bz and di == 1))
            if not bz:
                nc.tensor.matmul(out=lg[:],
                                 lhsT=C16("ones", rows=slice(0, 1))[:, :G_cap],
                                 rhs=C16("bl", rows=slice(0, 1)),
                                 start=False, stop=True)
            ez = spool.tile([G_cap, C], f32, name="ez", tag="ez")
            se = spool.tile([G_cap, 1], f32, name="se", tag="se")
            nc.scalar.activation(ez[:], lg[:], Exp, accum_out=se[:])
            lse = spool.tile([G_cap, 1], f32, name="lse", tag="lse")
            nc.scalar.activation(lse[:], se[:], Ln)
            res = spool.tile([G_cap, C], f32, name="res", tag="res")
            nc.vector.tensor_scalar(out=res[:], in0=lg[:], scalar1=lse[:],
                                    scalar2=None, op0=sub)
            # out DMA from the Act queue (res lands right after Act's ln, so
            # the wait barely blocks it) -- keeps the SP queue a pure input
            # stream so the next repetition's input DMAs issue while this rep
            # computes
            if repeat == 1:
                nc.sync.dma_start(out_d[:], res[:])
            else:
                nc.scalar.dma_start(out_d[:], res[:])

    nc.compile()
    return nc


_PROG_CACHE = {}


def _prepare_maps(inputs):
    in_maps, meta = _preprocess(inputs["x"], inputs["edge_index"],
                                inputs["batch"], inputs["num_graphs"])
    _pack_consts(in_maps, inputs, meta)
    return in_maps, meta


def _prog_key(meta):
    return (meta["F"], meta["H"], meta["C"], meta["U"], meta["V"], meta["W"],
            meta["G_cap"], meta["bz"])


def _prepare(inputs):
    in_maps, meta = _prepare_maps(inputs)
    key = _prog_key(meta)
    if key not in _PROG_CACHE:
        _PROG_CACHE[key] = _build_program(*key)
    return _PROG_CACHE[key], in_maps, meta


def kernel(**inputs):
    from concourse.bass_utils import run_bass_kernel_spmd

    nc, in_maps, meta = _prepare(inputs)
    res = run_bass_kernel_spmd(nc, in_maps, list(range(NCORES)))
    G = meta["G"]
    cog, gl = meta["core_of_graph"], meta["glocal"]
    out = np.empty((G, meta["C"]), np.float32)
    for g in range(G):
        out[g] = res.results[cog[g]]["out"][gl[g]]
    return out
